# revision 41
# baseline (speedup 1.0000x reference)
"""Multi-head attention (B=2, S=4096, D=512, H=8) on 8 trn2 NeuronCores.

Sharding: query-row data-parallel. Core c handles batch c//4; its 1024
query rows are eight interleaved 128-row blocks (global block 4*g + c%4)
so the causal structure is identical on every core. Each core:
  phase 1: PE-transposes its batch's X (bf16), projects K^T [D,S] and
           V [S,D] (full sequence, replicated across the 4 cores of a
           batch), transposes/projects Q^T for its own rows.
  phase 2: per 256-wide query chunk (2 q-blocks), flash-style attention
           fully on-chip: S^T = K^T.T @ Q^T on PE (bf16), exp on ACT,
           causal masking at 128x128 subblock granularity via a tiny
           per-core band-mask tile (all-ones/triangular/zero), P^T @
           V_aug on PE where V_aug carries a ones column so the softmax
           denominator falls out of the same matmul; normalization via
           fast-approx reciprocal + rank-1 PE broadcast.
  phase 3: output projection + bias, DMA out.
All matmul operands are bf16 (fp32r measured ~1.7x slower per row on
HW); accumulation stays fp32 in PSUM. No collectives; the host slices
inputs per core and concatenates outputs.
"""

import numpy as np

# Problem dims (hardcoded per contract)
B, S, D, H, PD = 2, 4096, 512, 8, 64
P = 128
NCORES = 8
CPB = 4           # cores per batch
QR = S // CPB     # 1024 query rows per core
NQB = QR // P     # 8 q-blocks of 128 per core
QC = 512          # attention q-chunk width (4 q-blocks)
NQC = QR // QC    # 2
JB = QC // P      # q-blocks per chunk
SC = 512          # sequence chunk in projection phase
NSC = S // SC     # 8
NKT = S // P      # 32 key subblocks of 128
DC = D // P       # 4 d-chunks of 128
HP = H // 2       # 4 head-pairs
HG = 4            # heads per attention group (PSUM budget)
NHG = H // HG     # 2

_prog_cache = {}
_ATTN_VARIANT = 1


def _build_tril():
    """Optimized build for the causal-mask case (the common one)."""
    import concourse.mybir as mybir
    import concourse.tile as tile
    from concourse import bacc
    from concourse.masks import make_identity

    f32 = mybir.dt.float32
    f32r = mybir.dt.float32r
    bf16 = mybir.dt.bfloat16
    Exp = mybir.ActivationFunctionType.Exp

    nc = bacc.Bacc(debug=False, target_bir_lowering=False)

    xb_d = nc.declare_dram_parameter("xb", [S, D], f32, isOutput=False)
    xq_d = nc.declare_dram_parameter("xq", [QR, D], f32, isOutput=False)
    wq_d = nc.declare_dram_parameter("wq", [P, DC, D], bf16, isOutput=False)
    wk_d = nc.declare_dram_parameter("wk", [P, DC, D], bf16, isOutput=False)
    wv_d = nc.declare_dram_parameter("wv", [P, DC, D], bf16, isOutput=False)
    wo_d = nc.declare_dram_parameter("wo", [P, DC, D], bf16, isOutput=False)
    bq_d = nc.declare_dram_parameter("bq", [P, DC], f32, isOutput=False)
    bk_d = nc.declare_dram_parameter("bk", [P, DC], f32, isOutput=False)
    bv_d = nc.declare_dram_parameter("bv", [P, D], f32, isOutput=False)
    bo_d = nc.declare_dram_parameter("bo", [P, D], f32, isOutput=False)
    # per-core causal band masks: bm[k, m, q] for k-subblock offset m within
    # a q-block's 4-subblock diagonal band (all-ones / triangular / zeros)
    bm_d = nc.declare_dram_parameter("bandmask", [P, 4, P], bf16, isOutput=False)
    ones_d = nc.declare_dram_parameter("ones", [P, NKT, H], bf16, isOutput=False)
    out_d = nc.declare_dram_parameter("out", [QR, D], f32, isOutput=True)

    with tile.TileContext(nc) as tc, nc.allow_low_precision(
            reason="bf16 matmul operands; fp32 PSUM accumulation"):
        with (
            tc.tile_pool(name="const", bufs=1) as constp,
            tc.tile_pool(name="big", bufs=1) as bigp,
            tc.tile_pool(name="work", bufs=2) as work,
        ):
            ident = constp.tile([P, P], f32, tag="ident")
            make_identity(nc, ident)
            ones_col = constp.tile([1, PD], bf16, tag="ones")
            nc.vector.memset(ones_col[:], 1.0)
            bm = constp.tile([P, 4, P], bf16, tag="bm")

            wq = constp.tile([P, DC, D], bf16, tag="wq")
            wk = constp.tile([P, DC, D], bf16, tag="wk")
            wv = constp.tile([P, DC, D], bf16, tag="wv")
            wo = constp.tile([P, DC, D], bf16, tag="wo")
            bq = constp.tile([P, DC], f32, tag="bq")
            bk = constp.tile([P, DC], f32, tag="bk")
            bv = constp.tile([P, D], f32, tag="bv")
            bo = constp.tile([P, D], f32, tag="bo")

            # K^T [d-in-headpair, hp, s] and V [k, kti, h, d(+ones)] in bf16
            kts = bigp.tile([P, HP, S], bf16, tag="kts")
            vts = bigp.tile([P, NKT, H, PD + 1], bf16, tag="vts")
            # Q^T for this core's rows (local block order)
            qt = bigp.tile([P, HP, NQB, P], bf16, tag="qt")

            # ---- phase 1: transpose X, project K^T, V, Q^T ----
            with tc.tile_pool(name="ps1", bufs=2, space="PSUM") as ps1:
                for sci in range(NSC):
                    xraw = work.tile([P, SC // P, D], f32, tag="xraw")
                    nc.sync.dma_start(
                        xraw[:],
                        xb_d[sci * SC:(sci + 1) * SC].rearrange(
                            "(rt p) d -> p rt d", p=P),
                    )
                    if sci == 0:
                        # const DMAs issue behind the first X chunk so the
                        # transposes start as early as possible
                        for sb_t, dr_t in [(wk, wk_d), (bk, bk_d),
                                           (wv, wv_d), (bv, bv_d),
                                           (wq, wq_d), (bq, bq_d),
                                           (wo, wo_d), (bo, bo_d)]:
                            nc.sync.dma_start(sb_t[:], dr_t[:])
                        nc.sync.dma_start(bm[:], bm_d[:])
                        nc.sync.dma_start(vts[:, :, :, PD:PD + 1],
                                          ones_d[:, :, :, None])
                    xt = work.tile([P, DC, SC], bf16, tag="xt")
                    for rt in range(SC // P):
                        pst = ps1.tile([P, D], f32, tag="pst")
                        for dc in range(DC):
                            nc.tensor.transpose(
                                pst[:, dc * P:(dc + 1) * P],
                                xraw[:, rt, dc * P:(dc + 1) * P],
                                ident[:],
                            )
                        nc.vector.tensor_copy(
                            out=xt[:, :, rt * P:(rt + 1) * P],
                            in_=pst[:].rearrange("p (dc j) -> p dc j", dc=DC),
                        )
                    for hp in range(HP):
                        psk = ps1.tile([P, SC], f32, tag="psk")
                        for dc in range(DC):
                            nc.tensor.matmul(
                                psk[:],
                                wk[:, dc, hp * P:(hp + 1) * P],
                                xt[:, dc, :],
                                start=(dc == 0), stop=(dc == DC - 1),
                            )
                        nc.scalar.add(kts[:, hp, sci * SC:(sci + 1) * SC],
                                      psk[:], bk[:, hp:hp + 1])
                    for rt in range(SC // P):
                        psv = ps1.tile([P, D], f32, tag="psv")
                        for dc in range(DC):
                            nc.tensor.matmul(
                                psv[:],
                                xt[:, dc, rt * P:(rt + 1) * P],
                                wv[:, dc, :],
                                start=(dc == 0), stop=(dc == DC - 1),
                            )
                        kti = sci * (SC // P) + rt
                        nc.vector.tensor_add(
                            out=vts[:, kti, :, 0:PD],
                            in0=psv[:].rearrange("p (h d) -> p h d", h=H),
                            in1=bv[:].rearrange("p (h d) -> p h d", h=H),
                        )

                # Q^T for this core's rows
                for sci in range(QR // SC):
                    xqraw = work.tile([P, SC // P, D], f32, tag="xraw")
                    nc.sync.dma_start(
                        xqraw[:],
                        xq_d[sci * SC:(sci + 1) * SC].rearrange(
                            "(rt p) d -> p rt d", p=P),
                    )
                    xqt = work.tile([P, DC, SC], bf16, tag="xt")
                    for rt in range(SC // P):
                        pst = ps1.tile([P, D], f32, tag="pst")
                        for dc in range(DC):
                            nc.tensor.transpose(
                                pst[:, dc * P:(dc + 1) * P],
                                xqraw[:, rt, dc * P:(dc + 1) * P],
                                ident[:],
                            )
                        nc.vector.tensor_copy(
                            out=xqt[:, :, rt * P:(rt + 1) * P],
                            in_=pst[:].rearrange("p (dc j) -> p dc j", dc=DC),
                        )
                    for hp in range(HP):
                        psq = ps1.tile([P, SC], f32, tag="psk")
                        for dc in range(DC):
                            nc.tensor.matmul(
                                psq[:],
                                wq[:, dc, hp * P:(hp + 1) * P],
                                xqt[:, dc, :],
                                start=(dc == 0), stop=(dc == DC - 1),
                            )
                        gb = sci * (SC // P)
                        nc.scalar.add(
                            qt[:, hp, gb:gb + SC // P, :],
                            psq[:].rearrange("p (g j) -> p g j", g=SC // P),
                            bq[:, hp:hp + 1])

            # ---- phase 2+3: attention + output projection per q-chunk ----
            # 4 passes of 2 heads (one head-pair each); adjacent QK matmuls at
            # partition offsets 0/64 run concurrently in PE row groups. The kc
            # loop is software-pipelined: QK/exp of kc issue before PV of
            # kc-1, so the PE never waits on the exp chain. pvs tag-sets
            # alternate per pass so pass p+1 overlaps pass p's normalization.
            with (
                tc.tile_pool(name="p2", bufs=2) as p2,
                tc.tile_pool(name="p2s", bufs=2) as p2s,
                tc.tile_pool(name="qkps", bufs=4, space="PSUM") as qkps,
                tc.tile_pool(name="pvps", bufs=1, space="PSUM") as pvps,
            ):
                for qc in range(NQC):
                    j0 = JB * qc             # first local q-block of chunk
                    kmax = 4 * j0 + 4 * JB   # exclusive k-subblock bound
                    attnT = p2.tile([P, DC, QC], bf16, tag="attnT")

                    def kc_ranges(kc):
                        # active q-cols [cr, QC); mask bm[m] on [cr, cr+128)
                        if kc < 4 * j0:
                            return 0, QC, None
                        jb = (kc - 4 * j0) // 4
                        cr = jb * P
                        return cr, QC - cr, (kc % 4, cr)

                    pending_normB = []
                    for hp in range(HP):     # pass = one head-pair
                        heads = [2 * hp, 2 * hp + 1]
                        ts = 2 * (hp % 2)    # alternating pvs tag-set
                        pvs = {h: pvps.tile([PD + 1, QC], f32,
                                            tag=f"pv{ts + h % 2}",
                                            name=f"pv{qc}_{h}")
                               for h in heads}
                        prev = None
                        for kc in range(kmax):
                            cr, w, mband = kc_ranges(kc)
                            pts = {}
                            for h in heads:
                                po = (h % 2) * PD
                                pss = qkps.tile([P, QC], f32, tag="qk")
                                nc.tensor.matmul(
                                    pss[:, cr:cr + w],
                                    kts[po:po + PD, h // 2,
                                        kc * P:(kc + 1) * P],
                                    qt[po:po + PD, h // 2,
                                       j0 + cr // P:j0 + JB, :],
                                    start=True, stop=True,
                                )
                                pts[h] = pss
                            if pending_normB:
                                for fn in pending_normB:
                                    fn()
                                pending_normB = []
                            for h in heads:
                                pss = pts[h]
                                pt = p2s.tile([P, QC], bf16,
                                              tag=f"pt{h % 2}")
                                nc.scalar.activation(pt[:, cr:cr + w],
                                                     pss[:, cr:cr + w],
                                                     Exp, scale=0.125)
                                if mband is not None:
                                    m, mc = mband
                                    pr = p2s.tile([P, P], bf16,
                                                  tag=f"pr{h % 2}",
                                                  name=f"pr{h % 2}")
                                    nc.vector.tensor_mul(
                                        out=pr[:], in0=pt[:, mc:mc + P],
                                        in1=bm[:, m, :])
                                    nc.vector.tensor_copy(
                                        out=pt[:, mc:mc + P], in_=pr[:])
                                pts[h] = pt
                            if prev is not None:
                                pcr, pw, pb, ppts = prev
                                for h in heads:
                                    nc.tensor.matmul(
                                        pvs[h][:, pcr:pcr + pw],
                                        vts[:, pb, h, :],
                                        ppts[h][:, pcr:pcr + pw],
                                        start=(pb == 0),
                                        stop=(pb == kmax - 1),
                                        skip_group_check=True,
                                    )
                            prev = (cr, w, kc, pts)
                        pcr, pw, pb, ppts = prev
                        for h in heads:
                            nc.tensor.matmul(
                                pvs[h][:, pcr:pcr + pw],
                                vts[:, pb, h, :],
                                ppts[h][:, pcr:pcr + pw],
                                start=(pb == 0), stop=(pb == kmax - 1),
                                skip_group_check=True,
                            )
                        # normalization: DVE part now, PE broadcast deferred
                        # into the next pass's kc loop
                        for h in heads:
                            den = p2s.tile([1, QC], f32, tag=f"den{h % 2}")
                            nc.vector.tensor_copy(out=den[:],
                                                  in_=pvs[h][PD:PD + 1, :])
                            recsb = p2s.tile([1, QC], f32, tag=f"rec{h % 2}")
                            nc.vector.reciprocal_approx_fast(
                                out=recsb[:], in_=den[:])
                            recb = p2s.tile([1, QC], bf16, tag=f"recb{h % 2}")
                            nc.vector.tensor_copy(out=recb[:], in_=recsb[:])

                            def normB(h=h, recb=recb, pvs_h=pvs[h]):
                                bcp = qkps.tile([P, QC], f32, tag="qk",
                                                name="bcp")
                                nc.tensor.matmul(
                                    bcp[0:PD, :], ones_col[:], recb[:],
                                    start=True, stop=True,
                                )
                                bcs = p2s.tile([PD, QC], f32, tag="bcs",
                                               name="bcs")
                                nc.vector.tensor_copy(out=bcs[:],
                                                      in_=bcp[0:PD, :])
                                po = (h % 2) * PD
                                nc.vector.tensor_mul(
                                    out=attnT[po:po + PD, h // 2, :],
                                    in0=pvs_h[0:PD, :],
                                    in1=bcs[:],
                                )
                            pending_normB.append(normB)
                    for fn in pending_normB:
                        fn()

                    # output projection for this q-chunk
                    for b2 in range(QC // P):
                        psf = qkps.tile([P, D], f32, tag="qk", name="psf")
                        for dc in range(DC):
                            nc.tensor.matmul(
                                psf[:],
                                attnT[:, dc, b2 * P:(b2 + 1) * P],
                                wo[:, dc, :],
                                start=(dc == 0), stop=(dc == DC - 1),
                            )
                        osb = p2s.tile([P, D], f32, tag="osb")
                        nc.vector.tensor_add(out=osb[:], in0=psf[:], in1=bo[:])
                        nc.sync.dma_start(
                            out_d[qc * QC + b2 * P:qc * QC + (b2 + 1) * P, :],
                            osb[:],
                        )
    nc.finalize()
    return nc


def _build_generic(mode: str):
    """Fallback build for non-causal masks (none / binary / additive)."""
    import concourse.mybir as mybir
    import concourse.tile as tile
    from concourse import bacc
    from concourse.masks import make_identity

    f32 = mybir.dt.float32
    f32r = mybir.dt.float32r
    bf16 = mybir.dt.bfloat16
    Exp = mybir.ActivationFunctionType.Exp
    Alu = mybir.AluOpType

    GQC = 512         # generic path q-chunk
    GNQC = QR // GQC

    nc = bacc.Bacc(debug=False, target_bir_lowering=False)

    xb = nc.declare_dram_parameter("xb", [S, D], f32, isOutput=False)
    xq = nc.declare_dram_parameter("xq", [QR, D], f32, isOutput=False)
    wq_d = nc.declare_dram_parameter("wq", [P, DC, D], f32r, isOutput=False)
    wk_d = nc.declare_dram_parameter("wk", [P, DC, D], f32r, isOutput=False)
    wv_d = nc.declare_dram_parameter("wv", [P, DC, D], f32r, isOutput=False)
    wo_d = nc.declare_dram_parameter("wo", [P, DC, D], f32r, isOutput=False)
    bq_d = nc.declare_dram_parameter("bq", [P, DC], f32, isOutput=False)
    bk_d = nc.declare_dram_parameter("bk", [P, DC], f32, isOutput=False)
    bv_d = nc.declare_dram_parameter("bv", [P, D], f32, isOutput=False)
    bo_d = nc.declare_dram_parameter("bo", [P, D], f32, isOutput=False)
    ones_d = nc.declare_dram_parameter("ones", [P, H], bf16, isOutput=False)
    onesr_d = nc.declare_dram_parameter("onesr", [1, PD], f32r, isOutput=False)
    if mode == "add":
        maskT_d = nc.declare_dram_parameter("maskT", [S, QR], f32, isOutput=False)
    elif mode == "bin":
        maskT_d = nc.declare_dram_parameter("maskT", [S, QR], bf16, isOutput=False)
    out_d = nc.declare_dram_parameter("out", [QR, D], f32, isOutput=True)

    with tile.TileContext(nc) as tc, nc.allow_low_precision(
            reason="float32r tiles are 4-byte fp32; PE rounds reads only"):
        with (
            tc.tile_pool(name="const", bufs=1) as constp,
            tc.tile_pool(name="kt", bufs=1) as ktp,
            tc.tile_pool(name="vt", bufs=1) as vtp,
            tc.tile_pool(name="work", bufs=2) as work,
        ):
            ident = constp.tile([P, P], f32, tag="ident")
            make_identity(nc, ident)
            ones_col = constp.tile([1, PD], f32r, tag="ones")
            nc.sync.dma_start(ones_col[:], onesr_d[:])

            wq = constp.tile([P, DC, D], f32r, tag="wq")
            wo = constp.tile([P, DC, D], f32r, tag="wo")
            bq = constp.tile([P, DC], f32, tag="bq")
            bo = constp.tile([P, D], f32, tag="bo")
            for sb_t, dr_t in [(wq, wq_d), (wo, wo_d), (bq, bq_d), (bo, bo_d)]:
                nc.sync.dma_start(sb_t[:], dr_t[:])

            kts = [ktp.tile([P, HP, SC], bf16, tag=f"kt{i}", name=f"kt{i}")
                   for i in range(NSC)]
            vts = [vtp.tile([P, H, PD + 1], bf16, tag=f"v{i}", name=f"v{i}")
                   for i in range(NKT)]
            for t in vts:
                nc.sync.dma_start(t[:, :, PD:PD + 1], ones_d[:, :, None])

            with (
                tc.tile_pool(name="p1w", bufs=1) as p1w,
                tc.tile_pool(name="ps1", bufs=2, space="PSUM") as ps1,
            ):
                wk = p1w.tile([P, DC, D], f32r, tag="wk")
                wv = p1w.tile([P, DC, D], f32r, tag="wv")
                bk = p1w.tile([P, DC], f32, tag="bk")
                bv = p1w.tile([P, D], f32, tag="bv")
                for sb_t, dr_t in [(wk, wk_d), (wv, wv_d), (bk, bk_d), (bv, bv_d)]:
                    nc.sync.dma_start(sb_t[:], dr_t[:])

                for sci in range(NSC):
                    xraw = work.tile([P, SC // P, D], f32, tag="xraw")
                    nc.sync.dma_start(
                        xraw[:],
                        xb[sci * SC:(sci + 1) * SC].rearrange(
                            "(rt p) d -> p rt d", p=P),
                    )
                    xt = work.tile([P, DC, SC], f32r, tag="xt")
                    for rt in range(SC // P):
                        pst = ps1.tile([P, D], f32, tag="tps")
                        for dc in range(DC):
                            nc.tensor.transpose(
                                pst[:, dc * P:(dc + 1) * P],
                                xraw[:, rt, dc * P:(dc + 1) * P],
                                ident[:],
                            )
                        nc.scalar.copy(
                            out=xt[:, :, rt * P:(rt + 1) * P],
                            in_=pst[:].rearrange("p (dc j) -> p dc j", dc=DC),
                        )
                    for hp in range(HP):
                        psk = ps1.tile([P, SC], f32, tag="kproj")
                        for dc in range(DC):
                            nc.tensor.matmul(
                                psk[:],
                                wk[:, dc, hp * P:(hp + 1) * P],
                                xt[:, dc, :],
                                start=(dc == 0), stop=(dc == DC - 1),
                            )
                        nc.scalar.add(kts[sci][:, hp, :], psk[:], bk[:, hp:hp + 1])
                    for rt in range(SC // P):
                        psv = ps1.tile([P, D], f32, tag="vproj")
                        for dc in range(DC):
                            nc.tensor.matmul(
                                psv[:],
                                xt[:, dc, rt * P:(rt + 1) * P],
                                wv[:, dc, :],
                                start=(dc == 0), stop=(dc == DC - 1),
                            )
                        kti = sci * (SC // P) + rt
                        nc.vector.tensor_add(
                            out=vts[kti][:, :, 0:PD],
                            in0=psv[:].rearrange("p (h d) -> p h d", h=H),
                            in1=bv[:].rearrange("p (h d) -> p h d", h=H),
                        )

            with (
                tc.tile_pool(name="p2", bufs=2) as p2,
                tc.tile_pool(name="p2s", bufs=3) as p2s,
                tc.tile_pool(name="p2a", bufs=1) as p2a,
                tc.tile_pool(name="qkps", bufs=4, space="PSUM") as qkps,
                tc.tile_pool(name="pvps", bufs=1, space="PSUM") as pvps,
            ):
                for qc in range(GNQC):
                    xqraw = work.tile([P, GQC // P, D], f32, tag="xraw")
                    nc.sync.dma_start(
                        xqraw[:],
                        xq[qc * GQC:(qc + 1) * GQC].rearrange(
                            "(rt p) d -> p rt d", p=P),
                    )
                    xqt = work.tile([P, DC, GQC], f32r, tag="xt")
                    for rt in range(GQC // P):
                        pst = qkps.tile([P, D], f32, tag="qk")
                        for dc in range(DC):
                            nc.tensor.transpose(
                                pst[:, dc * P:(dc + 1) * P],
                                xqraw[:, rt, dc * P:(dc + 1) * P],
                                ident[:],
                            )
                        nc.scalar.copy(
                            out=xqt[:, :, rt * P:(rt + 1) * P],
                            in_=pst[:].rearrange("p (dc j) -> p dc j", dc=DC),
                        )
                    qt = p2.tile([P, HP, GQC], bf16, tag="qt")
                    for hp in range(HP):
                        psq = qkps.tile([P, D], f32, tag="qk")
                        for dc in range(DC):
                            nc.tensor.matmul(
                                psq[:, 0:GQC],
                                wq[:, dc, hp * P:(hp + 1) * P],
                                xqt[:, dc, :],
                                start=(dc == 0), stop=(dc == DC - 1),
                            )
                        nc.scalar.add(qt[:, hp, :], psq[:, 0:GQC], bq[:, hp:hp + 1])

                    attnT = p2a.tile([P, DC, GQC], f32r, tag="attnT")
                    for hg in range(NHG):
                        heads = range(hg * HG, (hg + 1) * HG)
                        pvs = {h: pvps.tile([PD + 1, GQC], f32, tag=f"pv{h % HG}",
                                            name=f"pv{h}")
                               for h in heads}
                        for kc in range(NKT):
                            if mode == "add":
                                mt = p2s.tile([P, GQC], f32, tag="mt")
                            elif mode == "bin":
                                mt = p2s.tile([P, GQC], bf16, tag="mt")
                            if mode != "none":
                                nc.sync.dma_start(
                                    mt[:],
                                    maskT_d[kc * P:(kc + 1) * P,
                                            qc * GQC:(qc + 1) * GQC],
                                )
                            for h in heads:
                                po = (h % 2) * PD
                                pss = qkps.tile([P, D], f32, tag="qk")
                                nc.tensor.matmul(
                                    pss[:, 0:GQC],
                                    kts[kc // (SC // P)][
                                        po:po + PD, h // 2,
                                        (kc % (SC // P)) * P:
                                        (kc % (SC // P) + 1) * P],
                                    qt[po:po + PD, h // 2, :],
                                    start=True, stop=True,
                                )
                                pt = p2s.tile([P, GQC], bf16, tag="pt")
                                if mode == "add":
                                    st = p2s.tile([P, GQC], f32, tag="st")
                                    nc.vector.scalar_tensor_tensor(
                                        out=st[:], in0=mt[:], scalar=-1e9,
                                        in1=pss[:, 0:GQC],
                                        op0=Alu.mult, op1=Alu.add,
                                    )
                                    nc.scalar.activation(pt[:], st[:], Exp,
                                                         scale=0.125)
                                elif mode == "bin":
                                    pr = p2s.tile([P, GQC], bf16, tag="pr")
                                    nc.scalar.activation(pr[:], pss[:, 0:GQC], Exp,
                                                         scale=0.125)
                                    nc.vector.tensor_mul(
                                        out=pt[:], in0=pr[:], in1=mt[:])
                                else:
                                    nc.scalar.activation(pt[:], pss[:, 0:GQC], Exp,
                                                         scale=0.125)
                                nc.tensor.matmul(
                                    pvs[h][:],
                                    vts[kc][:, h, :],
                                    pt[:],
                                    start=(kc == 0), stop=(kc == NKT - 1),
                                    skip_group_check=True,
                                )
                        for h in heads:
                            recip = p2s.tile([1, GQC], f32r, tag="recip")
                            nc.vector.reciprocal(recip[:], pvs[h][PD:PD + 1, :])
                            bcp = fps.tile([PD, GQC], f32, tag="fin")
                            nc.tensor.matmul(
                                bcp[:], ones_col[:], recip[:],
                                start=True, stop=True,
                            )
                            bcs = p2s.tile([PD, GQC], f32, tag="bcs")
                            nc.vector.tensor_copy(out=bcs[:], in_=bcp[:])
                            po = (h % 2) * PD
                            nc.vector.tensor_mul(
                                out=attnT[po:po + PD, h // 2, :],
                                in0=pvs[h][0:PD, :],
                                in1=bcs[:],
                            )

                    for rt in range(GQC // P):
                        psf = qkps.tile([P, D], f32, tag="qk", name="psf")
                        for dc in range(DC):
                            nc.tensor.matmul(
                                psf[:],
                                attnT[:, dc, rt * P:(rt + 1) * P],
                                wo[:, dc, :],
                                start=(dc == 0), stop=(dc == DC - 1),
                            )
                        osb = p2s.tile([P, D], f32, tag="osb")
                        nc.vector.tensor_add(out=osb[:], in0=psf[:], in1=bo[:])
                        nc.sync.dma_start(
                            out_d[qc * GQC + rt * P: qc * GQC + (rt + 1) * P, :],
                            osb[:],
                        )
    nc.finalize()
    return nc


def _get_prog(mode: str):
    if mode not in _prog_cache:
        _prog_cache[mode] = (_build_tril() if mode == "tril"
                             else _build_generic(mode))
    return _prog_cache[mode]


def _q_rows(c, mode):
    """Query rows (into this core's batch) owned by core c."""
    if mode == "tril":
        # interleaved 128-row blocks so the causal kv range per q-chunk is
        # identical on every core
        j = np.arange(QR // P)
        base = (j * CPB + (c % CPB)) * P
        return (base[:, None] + np.arange(P)[None, :]).ravel()
    r0 = (c % CPB) * QR
    return np.arange(r0, r0 + QR)


def _warr(W, dtype):
    return np.ascontiguousarray(
        np.asarray(W, dtype=np.float32).reshape(DC, P, D)
        .transpose(1, 0, 2)).astype(dtype)


def _barr(b):
    return np.ascontiguousarray(
        np.asarray(b, dtype=np.float32).reshape(DC, P).T)


def make_in_maps(inputs, mask, Wq, bq, Wk, bk, Wv, bv, Wo, bo):
    import ml_dtypes
    bf = ml_dtypes.bfloat16
    inputs = np.asarray(inputs, dtype=np.float32)
    mask = np.asarray(mask, dtype=np.float32)
    if np.array_equal(mask, np.triu(np.ones((S, S), dtype=np.float32), 1)):
        mode = "tril"
    elif not np.any(mask):
        mode = "none"
    elif bool(((mask == 0.0) | (mask == 1.0)).all()):
        mode = "bin"
    else:
        mode = "add"

    in_maps = []
    if mode == "tril":
        shared = {
            "wq": _warr(Wq, bf), "wk": _warr(Wk, bf), "wv": _warr(Wv, bf),
            "wo": _warr(Wo, bf),
            "bq": _barr(bq), "bk": _barr(bk),
            "bv": np.ascontiguousarray(
                np.broadcast_to(np.asarray(bv, dtype=np.float32), (P, D))),
            "bo": np.ascontiguousarray(
                np.broadcast_to(np.asarray(bo, dtype=np.float32), (P, D))),
        }
        tri = np.triu(np.ones((P, P), dtype=np.float32))  # keep k <= q
        for c in range(NCORES):
            cl = c % CPB
            bmask = np.zeros((P, 4, P), dtype=np.float32)
            for m in range(4):
                if m < cl:
                    bmask[:, m, :] = 1.0
                elif m == cl:
                    bmask[:, m, :] = tri
            m = dict(shared)
            m["bandmask"] = np.ascontiguousarray(bmask).astype(bf)
            m["ones"] = np.ones((P, NKT, H), dtype=bf)
            bidx = c // CPB
            m["xb"] = np.ascontiguousarray(inputs[bidx])
            m["xq"] = np.ascontiguousarray(inputs[bidx][_q_rows(c, mode)])
            in_maps.append(m)
        return mode, in_maps

    # generic path (fp32/f32r)
    if mode == "none":
        maskT = None
    elif mode == "bin":
        maskT = np.ascontiguousarray(1.0 - mask.T).astype(bf)
    else:
        maskT = np.ascontiguousarray(mask.T)
    shared = {
        "wq": _warr(Wq, np.float32), "wk": _warr(Wk, np.float32),
        "wv": _warr(Wv, np.float32), "wo": _warr(Wo, np.float32),
        "bq": _barr(bq), "bk": _barr(bk),
        "bv": np.ascontiguousarray(
            np.broadcast_to(np.asarray(bv, dtype=np.float32), (P, D))),
        "bo": np.ascontiguousarray(
            np.broadcast_to(np.asarray(bo, dtype=np.float32), (P, D))),
    }
    for c in range(NCORES):
        b = c // CPB
        rows = _q_rows(c, mode)
        m = dict(shared)
        m["ones"] = np.ones((P, H), dtype=bf)
        m["onesr"] = np.ones((1, PD), dtype=np.float32)
        m["xb"] = np.ascontiguousarray(inputs[b])
        m["xq"] = np.ascontiguousarray(inputs[b][rows])
        if maskT is not None:
            m["maskT"] = np.ascontiguousarray(maskT[:, rows])
        in_maps.append(m)
    return mode, in_maps


def assemble(results, mode):
    out = np.empty((B, S, D), dtype=np.float32)
    for c in range(NCORES):
        b = c // CPB
        out[b, _q_rows(c, mode)] = results[c]["out"]
    return out


def kernel(inputs, mask, Wq, bq, Wk, bk, Wv, bv, Wo, bo):
    from concourse.bass_utils import run_bass_kernel_spmd

    mode, in_maps = make_in_maps(inputs, mask, Wq, bq, Wk, bk, Wv, bv, Wo, bo)
    nc = _get_prog(mode)
    res = run_bass_kernel_spmd(nc, in_maps, core_ids=list(range(NCORES)))
    return assemble(res.results, mode)


# revision 44
# speedup vs baseline: 1.1754x; 1.1754x over previous
"""Multi-head attention (B=2, S=4096, D=512, H=8) on 8 trn2 NeuronCores.

Sharding: query-row data-parallel. Core c handles batch c//4; its 1024
query rows are eight interleaved 128-row blocks (global block 4*g + c%4)
so the causal structure is identical on every core. Each core:
  phase 1: PE-transposes its batch's X (bf16), projects K^T [D,S] and
           V [S,D] (full sequence, replicated across the 4 cores of a
           batch), transposes/projects Q^T for its own rows.
  phase 2: per 256-wide query chunk (2 q-blocks), flash-style attention
           fully on-chip: S^T = K^T.T @ Q^T on PE (bf16), exp on ACT,
           causal masking at 128x128 subblock granularity via a tiny
           per-core band-mask tile (all-ones/triangular/zero), P^T @
           V_aug on PE where V_aug carries a ones column so the softmax
           denominator falls out of the same matmul; normalization via
           fast-approx reciprocal + rank-1 PE broadcast.
  phase 3: output projection + bias, DMA out.
All matmul operands are bf16 (fp32r measured ~1.7x slower per row on
HW); accumulation stays fp32 in PSUM. No collectives; the host slices
inputs per core and concatenates outputs.
"""

import numpy as np

# Problem dims (hardcoded per contract)
B, S, D, H, PD = 2, 4096, 512, 8, 64
P = 128
NCORES = 8
CPB = 4           # cores per batch
QR = S // CPB     # 1024 query rows per core
NQB = QR // P     # 8 q-blocks of 128 per core
QC = 512          # attention q-chunk width (4 q-blocks)
NQC = QR // QC    # 2
JB = QC // P      # q-blocks per chunk
SC = 512          # sequence chunk in projection phase
NSC = S // SC     # 8
NKT = S // P      # 32 key subblocks of 128
DC = D // P       # 4 d-chunks of 128
HP = H // 2       # 4 head-pairs
HG = 4            # heads per attention group (PSUM budget)
NHG = H // HG     # 2

_prog_cache = {}
_ATTN_VARIANT = 1


def _build_tril():
    """Optimized build for the causal-mask case (the common one)."""
    import concourse.mybir as mybir
    import concourse.tile as tile
    from concourse import bacc
    from concourse.masks import make_identity

    f32 = mybir.dt.float32
    f32r = mybir.dt.float32r
    bf16 = mybir.dt.bfloat16
    Exp = mybir.ActivationFunctionType.Exp

    nc = bacc.Bacc(debug=False, target_bir_lowering=False)

    xb_d = nc.declare_dram_parameter("xb", [S, D], f32, isOutput=False)
    xq_d = nc.declare_dram_parameter("xq", [QR, D], f32, isOutput=False)
    wq_d = nc.declare_dram_parameter("wq", [P, DC, D], bf16, isOutput=False)
    wk_d = nc.declare_dram_parameter("wk", [P, DC, D], bf16, isOutput=False)
    wv_d = nc.declare_dram_parameter("wv", [P, DC, D], bf16, isOutput=False)
    wo_d = nc.declare_dram_parameter("wo", [P, DC, D], bf16, isOutput=False)
    bq_d = nc.declare_dram_parameter("bq", [P, DC], f32, isOutput=False)
    bk_d = nc.declare_dram_parameter("bk", [P, DC], f32, isOutput=False)
    bv_d = nc.declare_dram_parameter("bv", [P, D], f32, isOutput=False)
    bo_d = nc.declare_dram_parameter("bo", [P, D], f32, isOutput=False)
    # per-core causal band masks: bm[k, m, q] for k-subblock offset m within
    # a q-block's 4-subblock diagonal band (all-ones / triangular / zeros)
    bm_d = nc.declare_dram_parameter("bandmask", [P, 4, P], bf16, isOutput=False)
    ones_d = nc.declare_dram_parameter("ones", [P, NKT, H], bf16, isOutput=False)
    out_d = nc.declare_dram_parameter("out", [QR, D], f32, isOutput=True)

    with tile.TileContext(nc) as tc, nc.allow_low_precision(
            reason="bf16 matmul operands; fp32 PSUM accumulation"):
        with (
            tc.tile_pool(name="const", bufs=1) as constp,
            tc.tile_pool(name="big", bufs=1) as bigp,
            tc.tile_pool(name="work", bufs=2) as work,
        ):
            ident = constp.tile([P, P], f32, tag="ident")
            make_identity(nc, ident)
            ones_col = constp.tile([1, PD], bf16, tag="ones")
            nc.vector.memset(ones_col[:], 1.0)
            bm = constp.tile([P, 4, P], bf16, tag="bm")

            wq = constp.tile([P, DC, D], bf16, tag="wq")
            wk = constp.tile([P, DC, D], bf16, tag="wk")
            wv = constp.tile([P, DC, D], bf16, tag="wv")
            wo = constp.tile([P, DC, D], bf16, tag="wo")
            bq = constp.tile([P, DC], f32, tag="bq")
            bk = constp.tile([P, DC], f32, tag="bk")
            bv = constp.tile([P, D], f32, tag="bv")
            bo = constp.tile([P, D], f32, tag="bo")

            # K^T [d-in-headpair, hp, s] and V [k, kti, h, d(+ones)] in bf16
            kts = bigp.tile([P, HP, S], bf16, tag="kts")
            vts = bigp.tile([P, NKT, H, PD + 1], bf16, tag="vts")
            # Q^T for this core's rows (local block order)
            qt = bigp.tile([P, HP, NQB, P], bf16, tag="qt")

            # ---- phase 1: transpose X, project K^T, V, Q^T ----
            with tc.tile_pool(name="ps1", bufs=2, space="PSUM") as ps1:
                for sci in range(NSC):
                    xraw = work.tile([P, SC // P, D], f32, tag="xraw")
                    nc.sync.dma_start(
                        xraw[:],
                        xb_d[sci * SC:(sci + 1) * SC].rearrange(
                            "(rt p) d -> p rt d", p=P),
                    )
                    if sci == 0:
                        # const DMAs issue behind the first X chunk so the
                        # transposes start as early as possible
                        for sb_t, dr_t in [(wk, wk_d), (bk, bk_d),
                                           (wv, wv_d), (bv, bv_d),
                                           (wq, wq_d), (bq, bq_d),
                                           (wo, wo_d), (bo, bo_d)]:
                            nc.sync.dma_start(sb_t[:], dr_t[:])
                        # slow many-descriptor DMAs go on the idle gpsimd
                        # queue so they never block the X-chunk stream
                        # (split to stay under the 16K-descriptor limit)
                        nc.gpsimd.dma_start(bm[:], bm_d[:])
                        for oq in range(4):
                            nc.gpsimd.dma_start(
                                vts[:, oq * 8:(oq + 1) * 8, :, PD:PD + 1],
                                ones_d[:, oq * 8:(oq + 1) * 8, :, None])
                    xt = work.tile([P, DC, SC], bf16, tag="xt")
                    for rt in range(SC // P):
                        pst = ps1.tile([P, D], f32, tag="pst")
                        for dc in range(DC):
                            nc.tensor.transpose(
                                pst[:, dc * P:(dc + 1) * P],
                                xraw[:, rt, dc * P:(dc + 1) * P],
                                ident[:],
                            )
                        nc.vector.tensor_copy(
                            out=xt[:, :, rt * P:(rt + 1) * P],
                            in_=pst[:].rearrange("p (dc j) -> p dc j", dc=DC),
                        )
                    for hp in range(HP):
                        psk = ps1.tile([P, SC], f32, tag="psk")
                        for dc in range(DC):
                            nc.tensor.matmul(
                                psk[:],
                                wk[:, dc, hp * P:(hp + 1) * P],
                                xt[:, dc, :],
                                start=(dc == 0), stop=(dc == DC - 1),
                            )
                        nc.scalar.add(kts[:, hp, sci * SC:(sci + 1) * SC],
                                      psk[:], bk[:, hp:hp + 1])
                    for rt in range(SC // P):
                        psv = ps1.tile([P, D], f32, tag="psv")
                        for dc in range(DC):
                            nc.tensor.matmul(
                                psv[:],
                                xt[:, dc, rt * P:(rt + 1) * P],
                                wv[:, dc, :],
                                start=(dc == 0), stop=(dc == DC - 1),
                            )
                        kti = sci * (SC // P) + rt
                        nc.vector.tensor_add(
                            out=vts[:, kti, :, 0:PD],
                            in0=psv[:].rearrange("p (h d) -> p h d", h=H),
                            in1=bv[:].rearrange("p (h d) -> p h d", h=H),
                        )

                # Q^T for this core's rows
                for sci in range(QR // SC):
                    xqraw = work.tile([P, SC // P, D], f32, tag="xraw")
                    nc.sync.dma_start(
                        xqraw[:],
                        xq_d[sci * SC:(sci + 1) * SC].rearrange(
                            "(rt p) d -> p rt d", p=P),
                    )
                    xqt = work.tile([P, DC, SC], bf16, tag="xt")
                    for rt in range(SC // P):
                        pst = ps1.tile([P, D], f32, tag="pst")
                        for dc in range(DC):
                            nc.tensor.transpose(
                                pst[:, dc * P:(dc + 1) * P],
                                xqraw[:, rt, dc * P:(dc + 1) * P],
                                ident[:],
                            )
                        nc.vector.tensor_copy(
                            out=xqt[:, :, rt * P:(rt + 1) * P],
                            in_=pst[:].rearrange("p (dc j) -> p dc j", dc=DC),
                        )
                    for hp in range(HP):
                        psq = ps1.tile([P, SC], f32, tag="psk")
                        for dc in range(DC):
                            nc.tensor.matmul(
                                psq[:],
                                wq[:, dc, hp * P:(hp + 1) * P],
                                xqt[:, dc, :],
                                start=(dc == 0), stop=(dc == DC - 1),
                            )
                        gb = sci * (SC // P)
                        nc.scalar.add(
                            qt[:, hp, gb:gb + SC // P, :],
                            psq[:].rearrange("p (g j) -> p g j", g=SC // P),
                            bq[:, hp:hp + 1])

            # ---- phase 2+3: attention + output projection per q-chunk ----
            # 4 passes of 2 heads (one head-pair each); adjacent QK matmuls at
            # partition offsets 0/64 run concurrently in PE row groups. The kc
            # loop is software-pipelined: QK/exp of kc issue before PV of
            # kc-1, so the PE never waits on the exp chain. pvs tag-sets
            # alternate per pass so pass p+1 overlaps pass p's normalization.
            with (
                tc.tile_pool(name="p2", bufs=2) as p2,
                tc.tile_pool(name="p2s", bufs=2) as p2s,
                tc.tile_pool(name="qkps", bufs=3, space="PSUM") as qkps,
                tc.tile_pool(name="pvps", bufs=1, space="PSUM") as pvps,
                tc.tile_pool(name="fps", bufs=1, space="PSUM") as fps,
            ):
                for qc in range(NQC):
                    j0 = JB * qc             # first local q-block of chunk
                    kmax = 4 * j0 + 4 * JB   # exclusive k-subblock bound
                    attnT = p2.tile([P, DC, QC], bf16, tag="attnT")

                    def kc_ranges(kc):
                        # active q-cols [cr, QC); mask bm[m] on [cr, cr+128)
                        if kc < 4 * j0:
                            return 0, QC, None
                        jb = (kc - 4 * j0) // 4
                        cr = jb * P
                        return cr, QC - cr, (kc % 4, cr)

                    pending_normB = []
                    for hp in range(HP):     # pass = one head-pair
                        heads = [2 * hp, 2 * hp + 1]
                        ts = 2 * (hp % 2)    # alternating pvs tag-set
                        pvs = {h: pvps.tile([PD + 1, QC], f32,
                                            tag=f"pv{ts + h % 2}",
                                            name=f"pv{qc}_{h}")
                               for h in heads}
                        prev = None
                        for kc in range(kmax):
                            cr, w, mband = kc_ranges(kc)
                            pts = {}
                            for h in heads:
                                po = (h % 2) * PD
                                pss = qkps.tile([P, QC], f32, tag="qk")
                                nc.tensor.matmul(
                                    pss[:, cr:cr + w],
                                    kts[po:po + PD, h // 2,
                                        kc * P:(kc + 1) * P],
                                    qt[po:po + PD, h // 2,
                                       j0 + cr // P:j0 + JB, :],
                                    start=True, stop=True,
                                )
                                pts[h] = pss
                            if pending_normB:
                                for fn in pending_normB:
                                    fn()
                                pending_normB = []
                            for h in heads:
                                pss = pts[h]
                                pt = p2s.tile([P, QC], bf16,
                                              tag=f"pt{h % 2}")
                                nc.scalar.activation(pt[:, cr:cr + w],
                                                     pss[:, cr:cr + w],
                                                     Exp, scale=0.125)
                                if mband is not None:
                                    m, mc = mband
                                    pr = p2s.tile([P, P], bf16,
                                                  tag=f"pr{h % 2}",
                                                  name=f"pr{h % 2}")
                                    nc.vector.tensor_mul(
                                        out=pr[:], in0=pt[:, mc:mc + P],
                                        in1=bm[:, m, :])
                                    nc.vector.tensor_copy(
                                        out=pt[:, mc:mc + P], in_=pr[:])
                                pts[h] = pt
                            if prev is not None:
                                pcr, pw, pb, ppts = prev
                                for h in heads:
                                    nc.tensor.matmul(
                                        pvs[h][:, pcr:pcr + pw],
                                        vts[:, pb, h, :],
                                        ppts[h][:, pcr:pcr + pw],
                                        start=(pb == 0),
                                        stop=(pb == kmax - 1),
                                        skip_group_check=True,
                                    )
                            prev = (cr, w, kc, pts)
                        pcr, pw, pb, ppts = prev
                        for h in heads:
                            nc.tensor.matmul(
                                pvs[h][:, pcr:pcr + pw],
                                vts[:, pb, h, :],
                                ppts[h][:, pcr:pcr + pw],
                                start=(pb == 0), stop=(pb == kmax - 1),
                                skip_group_check=True,
                            )
                        # normalization: DVE part now, PE broadcast deferred
                        # into the next pass's kc loop
                        for h in heads:
                            den = p2s.tile([1, QC], f32, tag=f"den{h % 2}")
                            nc.vector.tensor_copy(out=den[:],
                                                  in_=pvs[h][PD:PD + 1, :])
                            recsb = p2s.tile([1, QC], f32, tag=f"rec{h % 2}")
                            nc.vector.reciprocal_approx_fast(
                                out=recsb[:], in_=den[:])
                            recb = p2s.tile([1, QC], bf16, tag=f"recb{h % 2}")
                            nc.vector.tensor_copy(out=recb[:], in_=recsb[:])

                            def normB(h=h, recb=recb, pvs_h=pvs[h]):
                                bcp = qkps.tile([P, QC], f32, tag="qk",
                                                name="bcp")
                                nc.tensor.matmul(
                                    bcp[0:PD, :], ones_col[:], recb[:],
                                    start=True, stop=True,
                                )
                                bcs = p2s.tile([PD, QC], f32, tag="bcs",
                                               name="bcs")
                                nc.vector.tensor_copy(out=bcs[:],
                                                      in_=bcp[0:PD, :])
                                po = (h % 2) * PD
                                nc.vector.tensor_mul(
                                    out=attnT[po:po + PD, h // 2, :],
                                    in0=pvs_h[0:PD, :],
                                    in1=bcs[:],
                                )
                            pending_normB.append(normB)
                    for fn in pending_normB:
                        fn()

                    # output projection for this q-chunk
                    for b2 in range(QC // P):
                        psf = fps.tile([P, D], f32, tag="fin")
                        for dc in range(DC):
                            nc.tensor.matmul(
                                psf[:],
                                attnT[:, dc, b2 * P:(b2 + 1) * P],
                                wo[:, dc, :],
                                start=(dc == 0), stop=(dc == DC - 1),
                            )
                        osb = p2s.tile([P, D], f32, tag="osb")
                        nc.vector.tensor_add(out=osb[:], in0=psf[:], in1=bo[:])
                        nc.sync.dma_start(
                            out_d[qc * QC + b2 * P:qc * QC + (b2 + 1) * P, :],
                            osb[:],
                        )
    nc.finalize()
    return nc


def _build_generic(mode: str):
    """Fallback build for non-causal masks (none / binary / additive)."""
    import concourse.mybir as mybir
    import concourse.tile as tile
    from concourse import bacc
    from concourse.masks import make_identity

    f32 = mybir.dt.float32
    f32r = mybir.dt.float32r
    bf16 = mybir.dt.bfloat16
    Exp = mybir.ActivationFunctionType.Exp
    Alu = mybir.AluOpType

    GQC = 512         # generic path q-chunk
    GNQC = QR // GQC

    nc = bacc.Bacc(debug=False, target_bir_lowering=False)

    xb = nc.declare_dram_parameter("xb", [S, D], f32, isOutput=False)
    xq = nc.declare_dram_parameter("xq", [QR, D], f32, isOutput=False)
    wq_d = nc.declare_dram_parameter("wq", [P, DC, D], f32r, isOutput=False)
    wk_d = nc.declare_dram_parameter("wk", [P, DC, D], f32r, isOutput=False)
    wv_d = nc.declare_dram_parameter("wv", [P, DC, D], f32r, isOutput=False)
    wo_d = nc.declare_dram_parameter("wo", [P, DC, D], f32r, isOutput=False)
    bq_d = nc.declare_dram_parameter("bq", [P, DC], f32, isOutput=False)
    bk_d = nc.declare_dram_parameter("bk", [P, DC], f32, isOutput=False)
    bv_d = nc.declare_dram_parameter("bv", [P, D], f32, isOutput=False)
    bo_d = nc.declare_dram_parameter("bo", [P, D], f32, isOutput=False)
    ones_d = nc.declare_dram_parameter("ones", [P, H], bf16, isOutput=False)
    onesr_d = nc.declare_dram_parameter("onesr", [1, PD], f32r, isOutput=False)
    if mode == "add":
        maskT_d = nc.declare_dram_parameter("maskT", [S, QR], f32, isOutput=False)
    elif mode == "bin":
        maskT_d = nc.declare_dram_parameter("maskT", [S, QR], bf16, isOutput=False)
    out_d = nc.declare_dram_parameter("out", [QR, D], f32, isOutput=True)

    with tile.TileContext(nc) as tc, nc.allow_low_precision(
            reason="float32r tiles are 4-byte fp32; PE rounds reads only"):
        with (
            tc.tile_pool(name="const", bufs=1) as constp,
            tc.tile_pool(name="kt", bufs=1) as ktp,
            tc.tile_pool(name="vt", bufs=1) as vtp,
            tc.tile_pool(name="work", bufs=2) as work,
        ):
            ident = constp.tile([P, P], f32, tag="ident")
            make_identity(nc, ident)
            ones_col = constp.tile([1, PD], f32r, tag="ones")
            nc.sync.dma_start(ones_col[:], onesr_d[:])

            wq = constp.tile([P, DC, D], f32r, tag="wq")
            wo = constp.tile([P, DC, D], f32r, tag="wo")
            bq = constp.tile([P, DC], f32, tag="bq")
            bo = constp.tile([P, D], f32, tag="bo")
            for sb_t, dr_t in [(wq, wq_d), (wo, wo_d), (bq, bq_d), (bo, bo_d)]:
                nc.sync.dma_start(sb_t[:], dr_t[:])

            kts = [ktp.tile([P, HP, SC], bf16, tag=f"kt{i}", name=f"kt{i}")
                   for i in range(NSC)]
            vts = [vtp.tile([P, H, PD + 1], bf16, tag=f"v{i}", name=f"v{i}")
                   for i in range(NKT)]
            for t in vts:
                nc.sync.dma_start(t[:, :, PD:PD + 1], ones_d[:, :, None])

            with (
                tc.tile_pool(name="p1w", bufs=1) as p1w,
                tc.tile_pool(name="ps1", bufs=2, space="PSUM") as ps1,
            ):
                wk = p1w.tile([P, DC, D], f32r, tag="wk")
                wv = p1w.tile([P, DC, D], f32r, tag="wv")
                bk = p1w.tile([P, DC], f32, tag="bk")
                bv = p1w.tile([P, D], f32, tag="bv")
                for sb_t, dr_t in [(wk, wk_d), (wv, wv_d), (bk, bk_d), (bv, bv_d)]:
                    nc.sync.dma_start(sb_t[:], dr_t[:])

                for sci in range(NSC):
                    xraw = work.tile([P, SC // P, D], f32, tag="xraw")
                    nc.sync.dma_start(
                        xraw[:],
                        xb[sci * SC:(sci + 1) * SC].rearrange(
                            "(rt p) d -> p rt d", p=P),
                    )
                    xt = work.tile([P, DC, SC], f32r, tag="xt")
                    for rt in range(SC // P):
                        pst = ps1.tile([P, D], f32, tag="tps")
                        for dc in range(DC):
                            nc.tensor.transpose(
                                pst[:, dc * P:(dc + 1) * P],
                                xraw[:, rt, dc * P:(dc + 1) * P],
                                ident[:],
                            )
                        nc.scalar.copy(
                            out=xt[:, :, rt * P:(rt + 1) * P],
                            in_=pst[:].rearrange("p (dc j) -> p dc j", dc=DC),
                        )
                    for hp in range(HP):
                        psk = ps1.tile([P, SC], f32, tag="kproj")
                        for dc in range(DC):
                            nc.tensor.matmul(
                                psk[:],
                                wk[:, dc, hp * P:(hp + 1) * P],
                                xt[:, dc, :],
                                start=(dc == 0), stop=(dc == DC - 1),
                            )
                        nc.scalar.add(kts[sci][:, hp, :], psk[:], bk[:, hp:hp + 1])
                    for rt in range(SC // P):
                        psv = ps1.tile([P, D], f32, tag="vproj")
                        for dc in range(DC):
                            nc.tensor.matmul(
                                psv[:],
                                xt[:, dc, rt * P:(rt + 1) * P],
                                wv[:, dc, :],
                                start=(dc == 0), stop=(dc == DC - 1),
                            )
                        kti = sci * (SC // P) + rt
                        nc.vector.tensor_add(
                            out=vts[kti][:, :, 0:PD],
                            in0=psv[:].rearrange("p (h d) -> p h d", h=H),
                            in1=bv[:].rearrange("p (h d) -> p h d", h=H),
                        )

            with (
                tc.tile_pool(name="p2", bufs=2) as p2,
                tc.tile_pool(name="p2s", bufs=3) as p2s,
                tc.tile_pool(name="p2a", bufs=1) as p2a,
                tc.tile_pool(name="qkps", bufs=3, space="PSUM") as qkps,
                tc.tile_pool(name="pvps", bufs=1, space="PSUM") as pvps,
                tc.tile_pool(name="fps", bufs=1, space="PSUM") as fps,
            ):
                for qc in range(GNQC):
                    xqraw = work.tile([P, GQC // P, D], f32, tag="xraw")
                    nc.sync.dma_start(
                        xqraw[:],
                        xq[qc * GQC:(qc + 1) * GQC].rearrange(
                            "(rt p) d -> p rt d", p=P),
                    )
                    xqt = work.tile([P, DC, GQC], f32r, tag="xt")
                    for rt in range(GQC // P):
                        pst = qkps.tile([P, D], f32, tag="qk")
                        for dc in range(DC):
                            nc.tensor.transpose(
                                pst[:, dc * P:(dc + 1) * P],
                                xqraw[:, rt, dc * P:(dc + 1) * P],
                                ident[:],
                            )
                        nc.scalar.copy(
                            out=xqt[:, :, rt * P:(rt + 1) * P],
                            in_=pst[:].rearrange("p (dc j) -> p dc j", dc=DC),
                        )
                    qt = p2.tile([P, HP, GQC], bf16, tag="qt")
                    for hp in range(HP):
                        psq = qkps.tile([P, D], f32, tag="qk")
                        for dc in range(DC):
                            nc.tensor.matmul(
                                psq[:, 0:GQC],
                                wq[:, dc, hp * P:(hp + 1) * P],
                                xqt[:, dc, :],
                                start=(dc == 0), stop=(dc == DC - 1),
                            )
                        nc.scalar.add(qt[:, hp, :], psq[:, 0:GQC], bq[:, hp:hp + 1])

                    attnT = p2a.tile([P, DC, GQC], f32r, tag="attnT")
                    for hg in range(NHG):
                        heads = range(hg * HG, (hg + 1) * HG)
                        pvs = {h: pvps.tile([PD + 1, GQC], f32, tag=f"pv{h % HG}",
                                            name=f"pv{h}")
                               for h in heads}
                        for kc in range(NKT):
                            if mode == "add":
                                mt = p2s.tile([P, GQC], f32, tag="mt")
                            elif mode == "bin":
                                mt = p2s.tile([P, GQC], bf16, tag="mt")
                            if mode != "none":
                                nc.sync.dma_start(
                                    mt[:],
                                    maskT_d[kc * P:(kc + 1) * P,
                                            qc * GQC:(qc + 1) * GQC],
                                )
                            for h in heads:
                                po = (h % 2) * PD
                                pss = qkps.tile([P, D], f32, tag="qk")
                                nc.tensor.matmul(
                                    pss[:, 0:GQC],
                                    kts[kc // (SC // P)][
                                        po:po + PD, h // 2,
                                        (kc % (SC // P)) * P:
                                        (kc % (SC // P) + 1) * P],
                                    qt[po:po + PD, h // 2, :],
                                    start=True, stop=True,
                                )
                                pt = p2s.tile([P, GQC], bf16, tag="pt")
                                if mode == "add":
                                    st = p2s.tile([P, GQC], f32, tag="st")
                                    nc.vector.scalar_tensor_tensor(
                                        out=st[:], in0=mt[:], scalar=-1e9,
                                        in1=pss[:, 0:GQC],
                                        op0=Alu.mult, op1=Alu.add,
                                    )
                                    nc.scalar.activation(pt[:], st[:], Exp,
                                                         scale=0.125)
                                elif mode == "bin":
                                    pr = p2s.tile([P, GQC], bf16, tag="pr")
                                    nc.scalar.activation(pr[:], pss[:, 0:GQC], Exp,
                                                         scale=0.125)
                                    nc.vector.tensor_mul(
                                        out=pt[:], in0=pr[:], in1=mt[:])
                                else:
                                    nc.scalar.activation(pt[:], pss[:, 0:GQC], Exp,
                                                         scale=0.125)
                                nc.tensor.matmul(
                                    pvs[h][:],
                                    vts[kc][:, h, :],
                                    pt[:],
                                    start=(kc == 0), stop=(kc == NKT - 1),
                                    skip_group_check=True,
                                )
                        for h in heads:
                            recip = p2s.tile([1, GQC], f32r, tag="recip")
                            nc.vector.reciprocal(recip[:], pvs[h][PD:PD + 1, :])
                            bcp = fps.tile([PD, GQC], f32, tag="fin")
                            nc.tensor.matmul(
                                bcp[:], ones_col[:], recip[:],
                                start=True, stop=True,
                            )
                            bcs = p2s.tile([PD, GQC], f32, tag="bcs")
                            nc.vector.tensor_copy(out=bcs[:], in_=bcp[:])
                            po = (h % 2) * PD
                            nc.vector.tensor_mul(
                                out=attnT[po:po + PD, h // 2, :],
                                in0=pvs[h][0:PD, :],
                                in1=bcs[:],
                            )

                    for rt in range(GQC // P):
                        psf = fps.tile([P, D], f32, tag="fin")
                        for dc in range(DC):
                            nc.tensor.matmul(
                                psf[:],
                                attnT[:, dc, rt * P:(rt + 1) * P],
                                wo[:, dc, :],
                                start=(dc == 0), stop=(dc == DC - 1),
                            )
                        osb = p2s.tile([P, D], f32, tag="osb")
                        nc.vector.tensor_add(out=osb[:], in0=psf[:], in1=bo[:])
                        nc.sync.dma_start(
                            out_d[qc * GQC + rt * P: qc * GQC + (rt + 1) * P, :],
                            osb[:],
                        )
    nc.finalize()
    return nc


def _get_prog(mode: str):
    if mode not in _prog_cache:
        _prog_cache[mode] = (_build_tril() if mode == "tril"
                             else _build_generic(mode))
    return _prog_cache[mode]


def _q_rows(c, mode):
    """Query rows (into this core's batch) owned by core c."""
    if mode == "tril":
        # interleaved 128-row blocks so the causal kv range per q-chunk is
        # identical on every core
        j = np.arange(QR // P)
        base = (j * CPB + (c % CPB)) * P
        return (base[:, None] + np.arange(P)[None, :]).ravel()
    r0 = (c % CPB) * QR
    return np.arange(r0, r0 + QR)


def _warr(W, dtype):
    return np.ascontiguousarray(
        np.asarray(W, dtype=np.float32).reshape(DC, P, D)
        .transpose(1, 0, 2)).astype(dtype)


def _barr(b):
    return np.ascontiguousarray(
        np.asarray(b, dtype=np.float32).reshape(DC, P).T)


def make_in_maps(inputs, mask, Wq, bq, Wk, bk, Wv, bv, Wo, bo):
    import ml_dtypes
    bf = ml_dtypes.bfloat16
    inputs = np.asarray(inputs, dtype=np.float32)
    mask = np.asarray(mask, dtype=np.float32)
    if np.array_equal(mask, np.triu(np.ones((S, S), dtype=np.float32), 1)):
        mode = "tril"
    elif not np.any(mask):
        mode = "none"
    elif bool(((mask == 0.0) | (mask == 1.0)).all()):
        mode = "bin"
    else:
        mode = "add"

    in_maps = []
    if mode == "tril":
        shared = {
            "wq": _warr(Wq, bf), "wk": _warr(Wk, bf), "wv": _warr(Wv, bf),
            "wo": _warr(Wo, bf),
            "bq": _barr(bq), "bk": _barr(bk),
            "bv": np.ascontiguousarray(
                np.broadcast_to(np.asarray(bv, dtype=np.float32), (P, D))),
            "bo": np.ascontiguousarray(
                np.broadcast_to(np.asarray(bo, dtype=np.float32), (P, D))),
        }
        tri = np.triu(np.ones((P, P), dtype=np.float32))  # keep k <= q
        for c in range(NCORES):
            cl = c % CPB
            bmask = np.zeros((P, 4, P), dtype=np.float32)
            for m in range(4):
                if m < cl:
                    bmask[:, m, :] = 1.0
                elif m == cl:
                    bmask[:, m, :] = tri
            m = dict(shared)
            m["bandmask"] = np.ascontiguousarray(bmask).astype(bf)
            m["ones"] = np.ones((P, NKT, H), dtype=bf)
            bidx = c // CPB
            m["xb"] = np.ascontiguousarray(inputs[bidx])
            m["xq"] = np.ascontiguousarray(inputs[bidx][_q_rows(c, mode)])
            in_maps.append(m)
        return mode, in_maps

    # generic path (fp32/f32r)
    if mode == "none":
        maskT = None
    elif mode == "bin":
        maskT = np.ascontiguousarray(1.0 - mask.T).astype(bf)
    else:
        maskT = np.ascontiguousarray(mask.T)
    shared = {
        "wq": _warr(Wq, np.float32), "wk": _warr(Wk, np.float32),
        "wv": _warr(Wv, np.float32), "wo": _warr(Wo, np.float32),
        "bq": _barr(bq), "bk": _barr(bk),
        "bv": np.ascontiguousarray(
            np.broadcast_to(np.asarray(bv, dtype=np.float32), (P, D))),
        "bo": np.ascontiguousarray(
            np.broadcast_to(np.asarray(bo, dtype=np.float32), (P, D))),
    }
    for c in range(NCORES):
        b = c // CPB
        rows = _q_rows(c, mode)
        m = dict(shared)
        m["ones"] = np.ones((P, H), dtype=bf)
        m["onesr"] = np.ones((1, PD), dtype=np.float32)
        m["xb"] = np.ascontiguousarray(inputs[b])
        m["xq"] = np.ascontiguousarray(inputs[b][rows])
        if maskT is not None:
            m["maskT"] = np.ascontiguousarray(maskT[:, rows])
        in_maps.append(m)
    return mode, in_maps


def assemble(results, mode):
    out = np.empty((B, S, D), dtype=np.float32)
    for c in range(NCORES):
        b = c // CPB
        out[b, _q_rows(c, mode)] = results[c]["out"]
    return out


def kernel(inputs, mask, Wq, bq, Wk, bk, Wv, bv, Wo, bo):
    from concourse.bass_utils import run_bass_kernel_spmd

    mode, in_maps = make_in_maps(inputs, mask, Wq, bq, Wk, bk, Wv, bv, Wo, bo)
    nc = _get_prog(mode)
    res = run_bass_kernel_spmd(nc, in_maps, core_ids=list(range(NCORES)))
    return assemble(res.results, mode)


# revision 46
# speedup vs baseline: 1.2008x; 1.0216x over previous
"""Multi-head attention (B=2, S=4096, D=512, H=8) on 8 trn2 NeuronCores.

Sharding: query-row data-parallel. Core c handles batch c//4; its 1024
query rows are eight interleaved 128-row blocks (global block 4*g + c%4)
so the causal structure is identical on every core (SPMD: one program,
per-core differences live only in DMA'd data). Each core:
  phase 1: PE-transposes its batch's X, projects K^T [D,S] and V [S,D]
           (full sequence, replicated across the 4 cores of a batch),
           transposes/projects Q^T for its own rows. Slow many-descriptor
           const DMAs ride the gpsimd queue so the X-chunk stream on the
           sync queue is never blocked.
  phase 2: per 512-wide query chunk (4 q-blocks), flash-style attention
           fully on-chip, one head-pair per pass. S^T = K^T.T @ Q^T on PE
           (the pair's two K=64 matmuls sit at partition offsets 0/64 and
           execute concurrently in PE row groups), exp on ACT, P^T @
           V_aug on PE where V_aug carries a ones column so the softmax
           denominator falls out of the same matmul. Causal masking is
           subblock-ragged: matmul/exp N shrinks as whole 128-col q-blocks
           die, and the 4-subblock diagonal band multiplies a tiny
           per-core band-mask tile (all-ones/triangular/zero per core
           offset). The kc loop is software-pipelined (QK/exp of kc issue
           before PV of kc-1) so the PE never drains on the exp chain;
           normalization (approx-reciprocal + rank-1 PE broadcast) has
           its PE part deferred into the next pass's loop.
  phase 3: output projection + bias, DMA out.
All matmul operands are bf16 (fp32r measured ~1.7x slower per row on
HW); accumulation stays fp32 in PSUM. PSUM budget: 3 rotating QK banks +
4 PV accumulators (alternating tag-sets so pass p+1 overlaps pass p's
normalization) + 1 output-projection bank. No collectives; the host
slices inputs per core and concatenates outputs.

Hard-won HW constraints honored here (sim passes but HW fails if not):
  - two matmuls must not write disjoint ranges of one PSUM bank
    (start=True lazily zero-marks the whole 2KB bank),
  - ACT/DVE APs must not span PSUM bank boundaries,
  - DVE ops must not read and write the same SBUF range in-place,
  - custom-DVE ops (reciprocal_approx) read from SBUF, not PSUM.
"""

import numpy as np

# Problem dims (hardcoded per contract)
B, S, D, H, PD = 2, 4096, 512, 8, 64
P = 128
NCORES = 8
CPB = 4           # cores per batch
QR = S // CPB     # 1024 query rows per core
NQB = QR // P     # 8 q-blocks of 128 per core
QC = 512          # attention q-chunk width (4 q-blocks)
NQC = QR // QC    # 2
JB = QC // P      # q-blocks per chunk
SC = 512          # sequence chunk in projection phase
NSC = S // SC     # 8
NKT = S // P      # 32 key subblocks of 128
DC = D // P       # 4 d-chunks of 128
HP = H // 2       # 4 head-pairs
HG = 4            # heads per attention group (PSUM budget)
NHG = H // HG     # 2

_prog_cache = {}


def _build_tril():
    """Optimized build for the causal-mask case (the common one)."""
    import concourse.mybir as mybir
    import concourse.tile as tile
    from concourse import bacc
    from concourse.masks import make_identity

    f32 = mybir.dt.float32
    f32r = mybir.dt.float32r
    bf16 = mybir.dt.bfloat16
    Exp = mybir.ActivationFunctionType.Exp

    nc = bacc.Bacc(debug=False, target_bir_lowering=False)

    xb_d = nc.declare_dram_parameter("xb", [S, D], f32, isOutput=False)
    xq_d = nc.declare_dram_parameter("xq", [QR, D], f32, isOutput=False)
    wq_d = nc.declare_dram_parameter("wq", [P, DC, D], bf16, isOutput=False)
    wk_d = nc.declare_dram_parameter("wk", [P, DC, D], bf16, isOutput=False)
    wv_d = nc.declare_dram_parameter("wv", [P, DC, D], bf16, isOutput=False)
    wo_d = nc.declare_dram_parameter("wo", [P, DC, D], bf16, isOutput=False)
    bq_d = nc.declare_dram_parameter("bq", [P, DC], f32, isOutput=False)
    bk_d = nc.declare_dram_parameter("bk", [P, DC], f32, isOutput=False)
    bv_d = nc.declare_dram_parameter("bv", [P, D], f32, isOutput=False)
    bo_d = nc.declare_dram_parameter("bo", [P, D], f32, isOutput=False)
    # per-core causal band masks: bm[k, m, q] for k-subblock offset m within
    # a q-block's 4-subblock diagonal band (all-ones / triangular / zeros)
    bm_d = nc.declare_dram_parameter("bandmask", [P, 4, P], bf16, isOutput=False)
    ones_d = nc.declare_dram_parameter("ones", [P, NKT, H], bf16, isOutput=False)
    out_d = nc.declare_dram_parameter("out", [QR, D], f32, isOutput=True)

    with tile.TileContext(nc) as tc, nc.allow_low_precision(
            reason="bf16 matmul operands; fp32 PSUM accumulation"):
        with (
            tc.tile_pool(name="const", bufs=1) as constp,
            tc.tile_pool(name="big", bufs=1) as bigp,
            tc.tile_pool(name="work", bufs=2) as work,
        ):
            ident = constp.tile([P, P], f32, tag="ident")
            make_identity(nc, ident)
            ones_col = constp.tile([1, PD], bf16, tag="ones")
            nc.vector.memset(ones_col[:], 1.0)
            bm = constp.tile([P, 4, P], bf16, tag="bm")

            wq = constp.tile([P, DC, D], bf16, tag="wq")
            wk = constp.tile([P, DC, D], bf16, tag="wk")
            wv = constp.tile([P, DC, D], bf16, tag="wv")
            wo = constp.tile([P, DC, D], bf16, tag="wo")
            bq = constp.tile([P, DC], f32, tag="bq")
            bk = constp.tile([P, DC], f32, tag="bk")
            bv = constp.tile([P, D], f32, tag="bv")
            bo = constp.tile([P, D], f32, tag="bo")

            # K^T [d-in-headpair, hp, s] and V [k, kti, h, d(+ones)] in bf16
            kts = bigp.tile([P, HP, S], bf16, tag="kts")
            vts = bigp.tile([P, NKT, H, PD + 1], bf16, tag="vts")
            # Q^T for this core's rows (local block order)
            qt = bigp.tile([P, HP, NQB, P], bf16, tag="qt")

            # ---- phase 1: transpose X, project K^T, V, Q^T ----
            with tc.tile_pool(name="ps1", bufs=2, space="PSUM") as ps1:
                for sci in range(NSC):
                    xraw = work.tile([P, SC // P, D], f32, tag="xraw")
                    nc.sync.dma_start(
                        xraw[:],
                        xb_d[sci * SC:(sci + 1) * SC].rearrange(
                            "(rt p) d -> p rt d", p=P),
                    )
                    if sci == 0:
                        # const DMAs issue behind the first X chunk so the
                        # transposes start as early as possible
                        for sb_t, dr_t in [(wk, wk_d), (bk, bk_d),
                                           (wv, wv_d), (bv, bv_d),
                                           (wq, wq_d), (bq, bq_d),
                                           (wo, wo_d), (bo, bo_d)]:
                            nc.sync.dma_start(sb_t[:], dr_t[:])
                        # slow many-descriptor DMAs go on the idle gpsimd
                        # queue so they never block the X-chunk stream
                        # (split to stay under the 16K-descriptor limit)
                        nc.gpsimd.dma_start(bm[:], bm_d[:])
                        for oq in range(4):
                            nc.gpsimd.dma_start(
                                vts[:, oq * 8:(oq + 1) * 8, :, PD:PD + 1],
                                ones_d[:, oq * 8:(oq + 1) * 8, :, None])
                    xt = work.tile([P, DC, SC], bf16, tag="xt")
                    for rt in range(SC // P):
                        pst = ps1.tile([P, D], f32, tag="pst")
                        for dc in range(DC):
                            nc.tensor.transpose(
                                pst[:, dc * P:(dc + 1) * P],
                                xraw[:, rt, dc * P:(dc + 1) * P],
                                ident[:],
                            )
                        nc.vector.tensor_copy(
                            out=xt[:, :, rt * P:(rt + 1) * P],
                            in_=pst[:].rearrange("p (dc j) -> p dc j", dc=DC),
                        )
                    for hp in range(HP):
                        psk = ps1.tile([P, SC], f32, tag="psk")
                        for dc in range(DC):
                            nc.tensor.matmul(
                                psk[:],
                                wk[:, dc, hp * P:(hp + 1) * P],
                                xt[:, dc, :],
                                start=(dc == 0), stop=(dc == DC - 1),
                            )
                        nc.scalar.add(kts[:, hp, sci * SC:(sci + 1) * SC],
                                      psk[:], bk[:, hp:hp + 1])
                    for rt in range(SC // P):
                        psv = ps1.tile([P, D], f32, tag="psv")
                        for dc in range(DC):
                            nc.tensor.matmul(
                                psv[:],
                                xt[:, dc, rt * P:(rt + 1) * P],
                                wv[:, dc, :],
                                start=(dc == 0), stop=(dc == DC - 1),
                            )
                        kti = sci * (SC // P) + rt
                        nc.vector.tensor_add(
                            out=vts[:, kti, :, 0:PD],
                            in0=psv[:].rearrange("p (h d) -> p h d", h=H),
                            in1=bv[:].rearrange("p (h d) -> p h d", h=H),
                        )

                # Q^T for this core's rows
                for sci in range(QR // SC):
                    xqraw = work.tile([P, SC // P, D], f32, tag="xraw")
                    nc.sync.dma_start(
                        xqraw[:],
                        xq_d[sci * SC:(sci + 1) * SC].rearrange(
                            "(rt p) d -> p rt d", p=P),
                    )
                    xqt = work.tile([P, DC, SC], bf16, tag="xt")
                    for rt in range(SC // P):
                        pst = ps1.tile([P, D], f32, tag="pst")
                        for dc in range(DC):
                            nc.tensor.transpose(
                                pst[:, dc * P:(dc + 1) * P],
                                xqraw[:, rt, dc * P:(dc + 1) * P],
                                ident[:],
                            )
                        nc.vector.tensor_copy(
                            out=xqt[:, :, rt * P:(rt + 1) * P],
                            in_=pst[:].rearrange("p (dc j) -> p dc j", dc=DC),
                        )
                    for hp in range(HP):
                        psq = ps1.tile([P, SC], f32, tag="psk")
                        for dc in range(DC):
                            nc.tensor.matmul(
                                psq[:],
                                wq[:, dc, hp * P:(hp + 1) * P],
                                xqt[:, dc, :],
                                start=(dc == 0), stop=(dc == DC - 1),
                            )
                        gb = sci * (SC // P)
                        nc.scalar.add(
                            qt[:, hp, gb:gb + SC // P, :],
                            psq[:].rearrange("p (g j) -> p g j", g=SC // P),
                            bq[:, hp:hp + 1])

            # ---- phase 2+3: attention + output projection per q-chunk ----
            # 4 passes of 2 heads (one head-pair each); adjacent QK matmuls at
            # partition offsets 0/64 run concurrently in PE row groups. The kc
            # loop is software-pipelined: QK/exp of kc issue before PV of
            # kc-1, so the PE never waits on the exp chain. pvs tag-sets
            # alternate per pass so pass p+1 overlaps pass p's normalization.
            with (
                tc.tile_pool(name="p2", bufs=2) as p2,
                tc.tile_pool(name="p2s", bufs=2) as p2s,
                tc.tile_pool(name="qkps", bufs=3, space="PSUM") as qkps,
                tc.tile_pool(name="pvps", bufs=1, space="PSUM") as pvps,
                tc.tile_pool(name="fps", bufs=1, space="PSUM") as fps,
            ):
                for qc in range(NQC):
                    j0 = JB * qc             # first local q-block of chunk
                    kmax = 4 * j0 + 4 * JB   # exclusive k-subblock bound
                    attnT = p2.tile([P, DC, QC], bf16, tag="attnT")

                    def kc_ranges(kc):
                        # active q-cols [cr, QC); mask bm[m] on [cr, cr+128)
                        if kc < 4 * j0:
                            return 0, QC, None
                        jb = (kc - 4 * j0) // 4
                        cr = jb * P
                        return cr, QC - cr, (kc % 4, cr)

                    pending_normB = []
                    for hp in range(HP):     # pass = one head-pair
                        heads = [2 * hp, 2 * hp + 1]
                        ts = 2 * (hp % 2)    # alternating pvs tag-set
                        pvs = {h: pvps.tile([PD + 1, QC], f32,
                                            tag=f"pv{ts + h % 2}",
                                            name=f"pv{qc}_{h}")
                               for h in heads}
                        prev = None
                        for kc in range(kmax):
                            cr, w, mband = kc_ranges(kc)
                            pts = {}
                            for h in heads:
                                po = (h % 2) * PD
                                pss = qkps.tile([P, QC], f32, tag="qk")
                                nc.tensor.matmul(
                                    pss[:, cr:cr + w],
                                    kts[po:po + PD, h // 2,
                                        kc * P:(kc + 1) * P],
                                    qt[po:po + PD, h // 2,
                                       j0 + cr // P:j0 + JB, :],
                                    start=True, stop=True,
                                )
                                pts[h] = pss
                            if pending_normB:
                                for fn in pending_normB:
                                    fn()
                                pending_normB = []
                            for h in heads:
                                pss = pts[h]
                                pt = p2s.tile([P, QC], bf16,
                                              tag=f"pt{h % 2}")
                                nc.scalar.activation(pt[:, cr:cr + w],
                                                     pss[:, cr:cr + w],
                                                     Exp, scale=0.125)
                                if mband is not None:
                                    m, mc = mband
                                    pr = p2s.tile([P, P], bf16,
                                                  tag=f"pr{h % 2}",
                                                  name=f"pr{h % 2}")
                                    nc.vector.tensor_mul(
                                        out=pr[:], in0=pt[:, mc:mc + P],
                                        in1=bm[:, m, :])
                                    nc.vector.tensor_copy(
                                        out=pt[:, mc:mc + P], in_=pr[:])
                                pts[h] = pt
                            if prev is not None:
                                pcr, pw, pb, ppts = prev
                                for h in heads:
                                    nc.tensor.matmul(
                                        pvs[h][:, pcr:pcr + pw],
                                        vts[:, pb, h, :],
                                        ppts[h][:, pcr:pcr + pw],
                                        start=(pb == 0),
                                        stop=(pb == kmax - 1),
                                        skip_group_check=True,
                                    )
                            prev = (cr, w, kc, pts)
                        pcr, pw, pb, ppts = prev
                        for h in heads:
                            nc.tensor.matmul(
                                pvs[h][:, pcr:pcr + pw],
                                vts[:, pb, h, :],
                                ppts[h][:, pcr:pcr + pw],
                                start=(pb == 0), stop=(pb == kmax - 1),
                                skip_group_check=True,
                            )
                        # normalization: DVE part now, PE broadcast deferred
                        # into the next pass's kc loop
                        for h in heads:
                            den = p2s.tile([1, QC], f32, tag=f"den{h % 2}")
                            nc.vector.tensor_copy(out=den[:],
                                                  in_=pvs[h][PD:PD + 1, :])
                            recsb = p2s.tile([1, QC], f32, tag=f"rec{h % 2}")
                            nc.vector.reciprocal_approx_fast(
                                out=recsb[:], in_=den[:])
                            recb = p2s.tile([1, QC], bf16, tag=f"recb{h % 2}")
                            nc.vector.tensor_copy(out=recb[:], in_=recsb[:])

                            def normB(h=h, recb=recb, pvs_h=pvs[h]):
                                bcp = qkps.tile([P, QC], f32, tag="qk",
                                                name="bcp")
                                nc.tensor.matmul(
                                    bcp[0:PD, :], ones_col[:], recb[:],
                                    start=True, stop=True,
                                )
                                bcs = p2s.tile([PD, QC], f32, tag="bcs",
                                               name="bcs")
                                nc.vector.tensor_copy(out=bcs[:],
                                                      in_=bcp[0:PD, :])
                                po = (h % 2) * PD
                                nc.vector.tensor_mul(
                                    out=attnT[po:po + PD, h // 2, :],
                                    in0=pvs_h[0:PD, :],
                                    in1=bcs[:],
                                )
                            pending_normB.append(normB)
                    for fn in pending_normB:
                        fn()

                    # output projection for this q-chunk
                    for b2 in range(QC // P):
                        psf = fps.tile([P, D], f32, tag="fin")
                        for dc in range(DC):
                            nc.tensor.matmul(
                                psf[:],
                                attnT[:, dc, b2 * P:(b2 + 1) * P],
                                wo[:, dc, :],
                                start=(dc == 0), stop=(dc == DC - 1),
                            )
                        osb = p2s.tile([P, D], f32, tag="osb")
                        nc.vector.tensor_add(out=osb[:], in0=psf[:], in1=bo[:])
                        nc.sync.dma_start(
                            out_d[qc * QC + b2 * P:qc * QC + (b2 + 1) * P, :],
                            osb[:],
                        )
    nc.finalize()
    return nc


def _build_generic(mode: str):
    """Fallback build for non-causal masks (none / binary / additive)."""
    import concourse.mybir as mybir
    import concourse.tile as tile
    from concourse import bacc
    from concourse.masks import make_identity

    f32 = mybir.dt.float32
    f32r = mybir.dt.float32r
    bf16 = mybir.dt.bfloat16
    Exp = mybir.ActivationFunctionType.Exp
    Alu = mybir.AluOpType

    GQC = 512         # generic path q-chunk
    GNQC = QR // GQC

    nc = bacc.Bacc(debug=False, target_bir_lowering=False)

    xb = nc.declare_dram_parameter("xb", [S, D], f32, isOutput=False)
    xq = nc.declare_dram_parameter("xq", [QR, D], f32, isOutput=False)
    wq_d = nc.declare_dram_parameter("wq", [P, DC, D], f32r, isOutput=False)
    wk_d = nc.declare_dram_parameter("wk", [P, DC, D], f32r, isOutput=False)
    wv_d = nc.declare_dram_parameter("wv", [P, DC, D], f32r, isOutput=False)
    wo_d = nc.declare_dram_parameter("wo", [P, DC, D], f32r, isOutput=False)
    bq_d = nc.declare_dram_parameter("bq", [P, DC], f32, isOutput=False)
    bk_d = nc.declare_dram_parameter("bk", [P, DC], f32, isOutput=False)
    bv_d = nc.declare_dram_parameter("bv", [P, D], f32, isOutput=False)
    bo_d = nc.declare_dram_parameter("bo", [P, D], f32, isOutput=False)
    ones_d = nc.declare_dram_parameter("ones", [P, H], bf16, isOutput=False)
    onesr_d = nc.declare_dram_parameter("onesr", [1, PD], f32r, isOutput=False)
    if mode == "add":
        maskT_d = nc.declare_dram_parameter("maskT", [S, QR], f32, isOutput=False)
    elif mode == "bin":
        maskT_d = nc.declare_dram_parameter("maskT", [S, QR], bf16, isOutput=False)
    out_d = nc.declare_dram_parameter("out", [QR, D], f32, isOutput=True)

    with tile.TileContext(nc) as tc, nc.allow_low_precision(
            reason="float32r tiles are 4-byte fp32; PE rounds reads only"):
        with (
            tc.tile_pool(name="const", bufs=1) as constp,
            tc.tile_pool(name="kt", bufs=1) as ktp,
            tc.tile_pool(name="vt", bufs=1) as vtp,
            tc.tile_pool(name="work", bufs=2) as work,
        ):
            ident = constp.tile([P, P], f32, tag="ident")
            make_identity(nc, ident)
            ones_col = constp.tile([1, PD], f32r, tag="ones")
            nc.sync.dma_start(ones_col[:], onesr_d[:])

            wq = constp.tile([P, DC, D], f32r, tag="wq")
            wo = constp.tile([P, DC, D], f32r, tag="wo")
            bq = constp.tile([P, DC], f32, tag="bq")
            bo = constp.tile([P, D], f32, tag="bo")
            for sb_t, dr_t in [(wq, wq_d), (wo, wo_d), (bq, bq_d), (bo, bo_d)]:
                nc.sync.dma_start(sb_t[:], dr_t[:])

            kts = [ktp.tile([P, HP, SC], bf16, tag=f"kt{i}", name=f"kt{i}")
                   for i in range(NSC)]
            vts = [vtp.tile([P, H, PD + 1], bf16, tag=f"v{i}", name=f"v{i}")
                   for i in range(NKT)]
            for t in vts:
                nc.sync.dma_start(t[:, :, PD:PD + 1], ones_d[:, :, None])

            with (
                tc.tile_pool(name="p1w", bufs=1) as p1w,
                tc.tile_pool(name="ps1", bufs=2, space="PSUM") as ps1,
            ):
                wk = p1w.tile([P, DC, D], f32r, tag="wk")
                wv = p1w.tile([P, DC, D], f32r, tag="wv")
                bk = p1w.tile([P, DC], f32, tag="bk")
                bv = p1w.tile([P, D], f32, tag="bv")
                for sb_t, dr_t in [(wk, wk_d), (wv, wv_d), (bk, bk_d), (bv, bv_d)]:
                    nc.sync.dma_start(sb_t[:], dr_t[:])

                for sci in range(NSC):
                    xraw = work.tile([P, SC // P, D], f32, tag="xraw")
                    nc.sync.dma_start(
                        xraw[:],
                        xb[sci * SC:(sci + 1) * SC].rearrange(
                            "(rt p) d -> p rt d", p=P),
                    )
                    xt = work.tile([P, DC, SC], f32r, tag="xt")
                    for rt in range(SC // P):
                        pst = ps1.tile([P, D], f32, tag="tps")
                        for dc in range(DC):
                            nc.tensor.transpose(
                                pst[:, dc * P:(dc + 1) * P],
                                xraw[:, rt, dc * P:(dc + 1) * P],
                                ident[:],
                            )
                        nc.scalar.copy(
                            out=xt[:, :, rt * P:(rt + 1) * P],
                            in_=pst[:].rearrange("p (dc j) -> p dc j", dc=DC),
                        )
                    for hp in range(HP):
                        psk = ps1.tile([P, SC], f32, tag="kproj")
                        for dc in range(DC):
                            nc.tensor.matmul(
                                psk[:],
                                wk[:, dc, hp * P:(hp + 1) * P],
                                xt[:, dc, :],
                                start=(dc == 0), stop=(dc == DC - 1),
                            )
                        nc.scalar.add(kts[sci][:, hp, :], psk[:], bk[:, hp:hp + 1])
                    for rt in range(SC // P):
                        psv = ps1.tile([P, D], f32, tag="vproj")
                        for dc in range(DC):
                            nc.tensor.matmul(
                                psv[:],
                                xt[:, dc, rt * P:(rt + 1) * P],
                                wv[:, dc, :],
                                start=(dc == 0), stop=(dc == DC - 1),
                            )
                        kti = sci * (SC // P) + rt
                        nc.vector.tensor_add(
                            out=vts[kti][:, :, 0:PD],
                            in0=psv[:].rearrange("p (h d) -> p h d", h=H),
                            in1=bv[:].rearrange("p (h d) -> p h d", h=H),
                        )

            with (
                tc.tile_pool(name="p2", bufs=2) as p2,
                tc.tile_pool(name="p2s", bufs=3) as p2s,
                tc.tile_pool(name="p2a", bufs=1) as p2a,
                tc.tile_pool(name="qkps", bufs=3, space="PSUM") as qkps,
                tc.tile_pool(name="pvps", bufs=1, space="PSUM") as pvps,
                tc.tile_pool(name="fps", bufs=1, space="PSUM") as fps,
            ):
                for qc in range(GNQC):
                    xqraw = work.tile([P, GQC // P, D], f32, tag="xraw")
                    nc.sync.dma_start(
                        xqraw[:],
                        xq[qc * GQC:(qc + 1) * GQC].rearrange(
                            "(rt p) d -> p rt d", p=P),
                    )
                    xqt = work.tile([P, DC, GQC], f32r, tag="xt")
                    for rt in range(GQC // P):
                        pst = qkps.tile([P, D], f32, tag="qk")
                        for dc in range(DC):
                            nc.tensor.transpose(
                                pst[:, dc * P:(dc + 1) * P],
                                xqraw[:, rt, dc * P:(dc + 1) * P],
                                ident[:],
                            )
                        nc.scalar.copy(
                            out=xqt[:, :, rt * P:(rt + 1) * P],
                            in_=pst[:].rearrange("p (dc j) -> p dc j", dc=DC),
                        )
                    qt = p2.tile([P, HP, GQC], bf16, tag="qt")
                    for hp in range(HP):
                        psq = qkps.tile([P, D], f32, tag="qk")
                        for dc in range(DC):
                            nc.tensor.matmul(
                                psq[:, 0:GQC],
                                wq[:, dc, hp * P:(hp + 1) * P],
                                xqt[:, dc, :],
                                start=(dc == 0), stop=(dc == DC - 1),
                            )
                        nc.scalar.add(qt[:, hp, :], psq[:, 0:GQC], bq[:, hp:hp + 1])

                    attnT = p2a.tile([P, DC, GQC], f32r, tag="attnT")
                    for hg in range(NHG):
                        heads = range(hg * HG, (hg + 1) * HG)
                        pvs = {h: pvps.tile([PD + 1, GQC], f32, tag=f"pv{h % HG}",
                                            name=f"pv{h}")
                               for h in heads}
                        for kc in range(NKT):
                            if mode == "add":
                                mt = p2s.tile([P, GQC], f32, tag="mt")
                            elif mode == "bin":
                                mt = p2s.tile([P, GQC], bf16, tag="mt")
                            if mode != "none":
                                nc.sync.dma_start(
                                    mt[:],
                                    maskT_d[kc * P:(kc + 1) * P,
                                            qc * GQC:(qc + 1) * GQC],
                                )
                            for h in heads:
                                po = (h % 2) * PD
                                pss = qkps.tile([P, D], f32, tag="qk")
                                nc.tensor.matmul(
                                    pss[:, 0:GQC],
                                    kts[kc // (SC // P)][
                                        po:po + PD, h // 2,
                                        (kc % (SC // P)) * P:
                                        (kc % (SC // P) + 1) * P],
                                    qt[po:po + PD, h // 2, :],
                                    start=True, stop=True,
                                )
                                pt = p2s.tile([P, GQC], bf16, tag="pt")
                                if mode == "add":
                                    st = p2s.tile([P, GQC], f32, tag="st")
                                    nc.vector.scalar_tensor_tensor(
                                        out=st[:], in0=mt[:], scalar=-1e9,
                                        in1=pss[:, 0:GQC],
                                        op0=Alu.mult, op1=Alu.add,
                                    )
                                    nc.scalar.activation(pt[:], st[:], Exp,
                                                         scale=0.125)
                                elif mode == "bin":
                                    pr = p2s.tile([P, GQC], bf16, tag="pr")
                                    nc.scalar.activation(pr[:], pss[:, 0:GQC], Exp,
                                                         scale=0.125)
                                    nc.vector.tensor_mul(
                                        out=pt[:], in0=pr[:], in1=mt[:])
                                else:
                                    nc.scalar.activation(pt[:], pss[:, 0:GQC], Exp,
                                                         scale=0.125)
                                nc.tensor.matmul(
                                    pvs[h][:],
                                    vts[kc][:, h, :],
                                    pt[:],
                                    start=(kc == 0), stop=(kc == NKT - 1),
                                    skip_group_check=True,
                                )
                        for h in heads:
                            recip = p2s.tile([1, GQC], f32r, tag="recip")
                            nc.vector.reciprocal(recip[:], pvs[h][PD:PD + 1, :])
                            bcp = fps.tile([PD, GQC], f32, tag="fin")
                            nc.tensor.matmul(
                                bcp[:], ones_col[:], recip[:],
                                start=True, stop=True,
                            )
                            bcs = p2s.tile([PD, GQC], f32, tag="bcs")
                            nc.vector.tensor_copy(out=bcs[:], in_=bcp[:])
                            po = (h % 2) * PD
                            nc.vector.tensor_mul(
                                out=attnT[po:po + PD, h // 2, :],
                                in0=pvs[h][0:PD, :],
                                in1=bcs[:],
                            )

                    for rt in range(GQC // P):
                        psf = fps.tile([P, D], f32, tag="fin")
                        for dc in range(DC):
                            nc.tensor.matmul(
                                psf[:],
                                attnT[:, dc, rt * P:(rt + 1) * P],
                                wo[:, dc, :],
                                start=(dc == 0), stop=(dc == DC - 1),
                            )
                        osb = p2s.tile([P, D], f32, tag="osb")
                        nc.vector.tensor_add(out=osb[:], in0=psf[:], in1=bo[:])
                        nc.sync.dma_start(
                            out_d[qc * GQC + rt * P: qc * GQC + (rt + 1) * P, :],
                            osb[:],
                        )
    nc.finalize()
    return nc


def _get_prog(mode: str):
    if mode not in _prog_cache:
        _prog_cache[mode] = (_build_tril() if mode == "tril"
                             else _build_generic(mode))
    return _prog_cache[mode]


def _q_rows(c, mode):
    """Query rows (into this core's batch) owned by core c."""
    if mode == "tril":
        # interleaved 128-row blocks so the causal kv range per q-chunk is
        # identical on every core
        j = np.arange(QR // P)
        base = (j * CPB + (c % CPB)) * P
        return (base[:, None] + np.arange(P)[None, :]).ravel()
    r0 = (c % CPB) * QR
    return np.arange(r0, r0 + QR)


def _warr(W, dtype):
    return np.ascontiguousarray(
        np.asarray(W, dtype=np.float32).reshape(DC, P, D)
        .transpose(1, 0, 2)).astype(dtype)


def _barr(b):
    return np.ascontiguousarray(
        np.asarray(b, dtype=np.float32).reshape(DC, P).T)


def make_in_maps(inputs, mask, Wq, bq, Wk, bk, Wv, bv, Wo, bo):
    import ml_dtypes
    bf = ml_dtypes.bfloat16
    inputs = np.asarray(inputs, dtype=np.float32)
    mask = np.asarray(mask, dtype=np.float32)
    if np.array_equal(mask, np.triu(np.ones((S, S), dtype=np.float32), 1)):
        mode = "tril"
    elif not np.any(mask):
        mode = "none"
    elif bool(((mask == 0.0) | (mask == 1.0)).all()):
        mode = "bin"
    else:
        mode = "add"

    in_maps = []
    if mode == "tril":
        shared = {
            "wq": _warr(Wq, bf), "wk": _warr(Wk, bf), "wv": _warr(Wv, bf),
            "wo": _warr(Wo, bf),
            "bq": _barr(bq), "bk": _barr(bk),
            "bv": np.ascontiguousarray(
                np.broadcast_to(np.asarray(bv, dtype=np.float32), (P, D))),
            "bo": np.ascontiguousarray(
                np.broadcast_to(np.asarray(bo, dtype=np.float32), (P, D))),
        }
        tri = np.triu(np.ones((P, P), dtype=np.float32))  # keep k <= q
        for c in range(NCORES):
            cl = c % CPB
            bmask = np.zeros((P, 4, P), dtype=np.float32)
            for m in range(4):
                if m < cl:
                    bmask[:, m, :] = 1.0
                elif m == cl:
                    bmask[:, m, :] = tri
            m = dict(shared)
            m["bandmask"] = np.ascontiguousarray(bmask).astype(bf)
            m["ones"] = np.ones((P, NKT, H), dtype=bf)
            bidx = c // CPB
            m["xb"] = np.ascontiguousarray(inputs[bidx])
            m["xq"] = np.ascontiguousarray(inputs[bidx][_q_rows(c, mode)])
            in_maps.append(m)
        return mode, in_maps

    # generic path (fp32/f32r)
    if mode == "none":
        maskT = None
    elif mode == "bin":
        maskT = np.ascontiguousarray(1.0 - mask.T).astype(bf)
    else:
        maskT = np.ascontiguousarray(mask.T)
    shared = {
        "wq": _warr(Wq, np.float32), "wk": _warr(Wk, np.float32),
        "wv": _warr(Wv, np.float32), "wo": _warr(Wo, np.float32),
        "bq": _barr(bq), "bk": _barr(bk),
        "bv": np.ascontiguousarray(
            np.broadcast_to(np.asarray(bv, dtype=np.float32), (P, D))),
        "bo": np.ascontiguousarray(
            np.broadcast_to(np.asarray(bo, dtype=np.float32), (P, D))),
    }
    for c in range(NCORES):
        b = c // CPB
        rows = _q_rows(c, mode)
        m = dict(shared)
        m["ones"] = np.ones((P, H), dtype=bf)
        m["onesr"] = np.ones((1, PD), dtype=np.float32)
        m["xb"] = np.ascontiguousarray(inputs[b])
        m["xq"] = np.ascontiguousarray(inputs[b][rows])
        if maskT is not None:
            m["maskT"] = np.ascontiguousarray(maskT[:, rows])
        in_maps.append(m)
    return mode, in_maps


def assemble(results, mode):
    out = np.empty((B, S, D), dtype=np.float32)
    for c in range(NCORES):
        b = c // CPB
        out[b, _q_rows(c, mode)] = results[c]["out"]
    return out


def kernel(inputs, mask, Wq, bq, Wk, bk, Wv, bv, Wo, bo):
    from concourse.bass_utils import run_bass_kernel_spmd

    mode, in_maps = make_in_maps(inputs, mask, Wq, bq, Wk, bk, Wv, bv, Wo, bo)
    nc = _get_prog(mode)
    res = run_bass_kernel_spmd(nc, in_maps, core_ids=list(range(NCORES)))
    return assemble(res.results, mode)


# revision 47
# speedup vs baseline: 1.2224x; 1.0180x over previous
"""Multi-head attention (B=2, S=4096, D=512, H=8) on 8 trn2 NeuronCores.

Sharding: query-row data-parallel. Core c handles batch c//4; its 1024
query rows are eight interleaved 128-row blocks (global block 4*g + c%4)
so the causal structure is identical on every core (SPMD: one program,
per-core differences live only in DMA'd data). Each core:
  phase 1: PE-transposes its batch's X, projects K^T [D,S] and V [S,D]
           (full sequence, replicated across the 4 cores of a batch),
           transposes/projects Q^T for its own rows. Slow many-descriptor
           const DMAs ride the gpsimd queue so the X-chunk stream on the
           sync queue is never blocked.
  phase 2: per 512-wide query chunk (4 q-blocks), flash-style attention
           fully on-chip, one head-pair per pass. S^T = K^T.T @ Q^T on PE
           (the pair's two K=64 matmuls sit at partition offsets 0/64 and
           execute concurrently in PE row groups), exp on ACT, P^T @
           V_aug on PE where V_aug carries a ones column so the softmax
           denominator falls out of the same matmul. Causal masking is
           subblock-ragged: matmul/exp N shrinks as whole 128-col q-blocks
           die, and the 4-subblock diagonal band multiplies a tiny
           per-core band-mask tile (all-ones/triangular/zero per core
           offset). The kc loop is software-pipelined (QK/exp of kc issue
           before PV of kc-1) so the PE never drains on the exp chain;
           normalization (approx-reciprocal + rank-1 PE broadcast) has
           its PE part deferred into the next pass's loop.
  phase 3: output projection + bias, DMA out.
All matmul operands are bf16 (fp32r measured ~1.7x slower per row on
HW); accumulation stays fp32 in PSUM. PSUM budget: 3 rotating QK banks +
4 PV accumulators (alternating tag-sets so pass p+1 overlaps pass p's
normalization) + 1 output-projection bank. No collectives; the host
slices inputs per core and concatenates outputs.

Hard-won HW constraints honored here (sim passes but HW fails if not):
  - two matmuls must not write disjoint ranges of one PSUM bank
    (start=True lazily zero-marks the whole 2KB bank),
  - ACT/DVE APs must not span PSUM bank boundaries,
  - DVE ops must not read and write the same SBUF range in-place,
  - custom-DVE ops (reciprocal_approx) read from SBUF, not PSUM.
"""

import numpy as np

# Problem dims (hardcoded per contract)
B, S, D, H, PD = 2, 4096, 512, 8, 64
P = 128
NCORES = 8
CPB = 4           # cores per batch
QR = S // CPB     # 1024 query rows per core
NQB = QR // P     # 8 q-blocks of 128 per core
QC = 512          # attention q-chunk width (4 q-blocks)
NQC = QR // QC    # 2
JB = QC // P      # q-blocks per chunk
SC = 512          # sequence chunk in projection phase
NSC = S // SC     # 8
NKT = S // P      # 32 key subblocks of 128
DC = D // P       # 4 d-chunks of 128
HP = H // 2       # 4 head-pairs
HG = 4            # heads per attention group (PSUM budget)
NHG = H // HG     # 2

_prog_cache = {}


def _build_tril():
    """Optimized build for the causal-mask case (the common one)."""
    import concourse.mybir as mybir
    import concourse.tile as tile
    from concourse import bacc
    from concourse.masks import make_identity

    f32 = mybir.dt.float32
    f32r = mybir.dt.float32r
    bf16 = mybir.dt.bfloat16
    Exp = mybir.ActivationFunctionType.Exp

    nc = bacc.Bacc(debug=False, target_bir_lowering=False)

    xb_d = nc.declare_dram_parameter("xb", [S, D], f32, isOutput=False)
    xq_d = nc.declare_dram_parameter("xq", [QR, D], f32, isOutput=False)
    wq_d = nc.declare_dram_parameter("wq", [P, DC, D], bf16, isOutput=False)
    wk_d = nc.declare_dram_parameter("wk", [P, DC, D], bf16, isOutput=False)
    wv_d = nc.declare_dram_parameter("wv", [P, DC, D], bf16, isOutput=False)
    wo_d = nc.declare_dram_parameter("wo", [P, DC, D], bf16, isOutput=False)
    bq_d = nc.declare_dram_parameter("bq", [P, DC], f32, isOutput=False)
    bk_d = nc.declare_dram_parameter("bk", [P, DC], f32, isOutput=False)
    bv_d = nc.declare_dram_parameter("bv", [P, D], f32, isOutput=False)
    bo_d = nc.declare_dram_parameter("bo", [P, D], f32, isOutput=False)
    # per-core causal band masks: bm[k, m, q] for k-subblock offset m within
    # a q-block's 4-subblock diagonal band (all-ones / triangular / zeros)
    bm_d = nc.declare_dram_parameter("bandmask", [P, 4, P], bf16, isOutput=False)
    ones_d = nc.declare_dram_parameter("ones", [P, NKT, H], bf16, isOutput=False)
    out_d = nc.declare_dram_parameter("out", [QR, D], f32, isOutput=True)

    with tile.TileContext(nc) as tc, nc.allow_low_precision(
            reason="bf16 matmul operands; fp32 PSUM accumulation"):
        with (
            tc.tile_pool(name="const", bufs=1) as constp,
            tc.tile_pool(name="big", bufs=1) as bigp,
            tc.tile_pool(name="work", bufs=3) as work,
        ):
            ident = constp.tile([P, P], f32, tag="ident")
            make_identity(nc, ident)
            ones_col = constp.tile([1, PD], bf16, tag="ones")
            nc.vector.memset(ones_col[:], 1.0)
            bm = constp.tile([P, 4, P], bf16, tag="bm")

            wq = constp.tile([P, DC, D], bf16, tag="wq")
            wk = constp.tile([P, DC, D], bf16, tag="wk")
            wv = constp.tile([P, DC, D], bf16, tag="wv")
            wo = constp.tile([P, DC, D], bf16, tag="wo")
            bq = constp.tile([P, DC], f32, tag="bq")
            bk = constp.tile([P, DC], f32, tag="bk")
            bv = constp.tile([P, D], f32, tag="bv")
            bo = constp.tile([P, D], f32, tag="bo")

            # K^T [d-in-headpair, hp, s] and V [k, kti, h, d(+ones)] in bf16
            kts = bigp.tile([P, HP, S], bf16, tag="kts")
            vts = bigp.tile([P, NKT, H, PD + 1], bf16, tag="vts")
            # Q^T for this core's rows (local block order)
            qt = bigp.tile([P, HP, NQB, P], bf16, tag="qt")

            # ---- phase 1: transpose X, project K^T, V, Q^T ----
            with tc.tile_pool(name="ps1", bufs=2, space="PSUM") as ps1:
                for sci in range(NSC):
                    xraw = work.tile([P, SC // P, D], f32, tag="xraw")
                    nc.sync.dma_start(
                        xraw[:],
                        xb_d[sci * SC:(sci + 1) * SC].rearrange(
                            "(rt p) d -> p rt d", p=P),
                    )
                    if sci == 0:
                        # const DMAs issue behind the first X chunk so the
                        # transposes start as early as possible
                        for sb_t, dr_t in [(wk, wk_d), (bk, bk_d),
                                           (wv, wv_d), (bv, bv_d),
                                           (wq, wq_d), (bq, bq_d),
                                           (wo, wo_d), (bo, bo_d)]:
                            nc.sync.dma_start(sb_t[:], dr_t[:])
                        # slow many-descriptor DMAs go on the idle gpsimd
                        # queue so they never block the X-chunk stream
                        # (split to stay under the 16K-descriptor limit)
                        nc.gpsimd.dma_start(bm[:], bm_d[:])
                        for oq in range(4):
                            nc.gpsimd.dma_start(
                                vts[:, oq * 8:(oq + 1) * 8, :, PD:PD + 1],
                                ones_d[:, oq * 8:(oq + 1) * 8, :, None])
                    xt = work.tile([P, DC, SC], bf16, tag="xt")
                    for rt in range(SC // P):
                        pst = ps1.tile([P, D], f32, tag="pst")
                        for dc in range(DC):
                            nc.tensor.transpose(
                                pst[:, dc * P:(dc + 1) * P],
                                xraw[:, rt, dc * P:(dc + 1) * P],
                                ident[:],
                            )
                        nc.vector.tensor_copy(
                            out=xt[:, :, rt * P:(rt + 1) * P],
                            in_=pst[:].rearrange("p (dc j) -> p dc j", dc=DC),
                        )
                    for hp in range(HP):
                        psk = ps1.tile([P, SC], f32, tag="psk")
                        for dc in range(DC):
                            nc.tensor.matmul(
                                psk[:],
                                wk[:, dc, hp * P:(hp + 1) * P],
                                xt[:, dc, :],
                                start=(dc == 0), stop=(dc == DC - 1),
                            )
                        nc.scalar.add(kts[:, hp, sci * SC:(sci + 1) * SC],
                                      psk[:], bk[:, hp:hp + 1])
                    for rt in range(SC // P):
                        psv = ps1.tile([P, D], f32, tag="psv")
                        for dc in range(DC):
                            nc.tensor.matmul(
                                psv[:],
                                xt[:, dc, rt * P:(rt + 1) * P],
                                wv[:, dc, :],
                                start=(dc == 0), stop=(dc == DC - 1),
                            )
                        kti = sci * (SC // P) + rt
                        nc.vector.tensor_add(
                            out=vts[:, kti, :, 0:PD],
                            in0=psv[:].rearrange("p (h d) -> p h d", h=H),
                            in1=bv[:].rearrange("p (h d) -> p h d", h=H),
                        )

                # Q^T for this core's rows
                for sci in range(QR // SC):
                    xqraw = work.tile([P, SC // P, D], f32, tag="xraw")
                    nc.sync.dma_start(
                        xqraw[:],
                        xq_d[sci * SC:(sci + 1) * SC].rearrange(
                            "(rt p) d -> p rt d", p=P),
                    )
                    xqt = work.tile([P, DC, SC], bf16, tag="xt")
                    for rt in range(SC // P):
                        pst = ps1.tile([P, D], f32, tag="pst")
                        for dc in range(DC):
                            nc.tensor.transpose(
                                pst[:, dc * P:(dc + 1) * P],
                                xqraw[:, rt, dc * P:(dc + 1) * P],
                                ident[:],
                            )
                        nc.vector.tensor_copy(
                            out=xqt[:, :, rt * P:(rt + 1) * P],
                            in_=pst[:].rearrange("p (dc j) -> p dc j", dc=DC),
                        )
                    for hp in range(HP):
                        psq = ps1.tile([P, SC], f32, tag="psk")
                        for dc in range(DC):
                            nc.tensor.matmul(
                                psq[:],
                                wq[:, dc, hp * P:(hp + 1) * P],
                                xqt[:, dc, :],
                                start=(dc == 0), stop=(dc == DC - 1),
                            )
                        gb = sci * (SC // P)
                        nc.scalar.add(
                            qt[:, hp, gb:gb + SC // P, :],
                            psq[:].rearrange("p (g j) -> p g j", g=SC // P),
                            bq[:, hp:hp + 1])

            # ---- phase 2+3: attention + output projection per q-chunk ----
            # 4 passes of 2 heads (one head-pair each); adjacent QK matmuls at
            # partition offsets 0/64 run concurrently in PE row groups. The kc
            # loop is software-pipelined: QK/exp of kc issue before PV of
            # kc-1, so the PE never waits on the exp chain. pvs tag-sets
            # alternate per pass so pass p+1 overlaps pass p's normalization.
            with (
                tc.tile_pool(name="p2", bufs=2) as p2,
                tc.tile_pool(name="p2s", bufs=2) as p2s,
                tc.tile_pool(name="qkps", bufs=3, space="PSUM") as qkps,
                tc.tile_pool(name="pvps", bufs=1, space="PSUM") as pvps,
                tc.tile_pool(name="fps", bufs=1, space="PSUM") as fps,
            ):
                for qc in range(NQC):
                    j0 = JB * qc             # first local q-block of chunk
                    kmax = 4 * j0 + 4 * JB   # exclusive k-subblock bound
                    attnT = p2.tile([P, DC, QC], bf16, tag="attnT")

                    def kc_ranges(kc):
                        # active q-cols [cr, QC); mask bm[m] on [cr, cr+128)
                        if kc < 4 * j0:
                            return 0, QC, None
                        jb = (kc - 4 * j0) // 4
                        cr = jb * P
                        return cr, QC - cr, (kc % 4, cr)

                    pending_normB = []
                    for hp in range(HP):     # pass = one head-pair
                        heads = [2 * hp, 2 * hp + 1]
                        ts = 2 * (hp % 2)    # alternating pvs tag-set
                        pvs = {h: pvps.tile([PD + 1, QC], f32,
                                            tag=f"pv{ts + h % 2}",
                                            name=f"pv{qc}_{h}")
                               for h in heads}
                        prev = None
                        for kc in range(kmax):
                            cr, w, mband = kc_ranges(kc)
                            pts = {}
                            for h in heads:
                                po = (h % 2) * PD
                                pss = qkps.tile([P, QC], f32, tag="qk")
                                nc.tensor.matmul(
                                    pss[:, cr:cr + w],
                                    kts[po:po + PD, h // 2,
                                        kc * P:(kc + 1) * P],
                                    qt[po:po + PD, h // 2,
                                       j0 + cr // P:j0 + JB, :],
                                    start=True, stop=True,
                                )
                                pts[h] = pss
                            if pending_normB:
                                for fn in pending_normB:
                                    fn()
                                pending_normB = []
                            for h in heads:
                                pss = pts[h]
                                pt = p2s.tile([P, QC], bf16,
                                              tag=f"pt{h % 2}")
                                nc.scalar.activation(pt[:, cr:cr + w],
                                                     pss[:, cr:cr + w],
                                                     Exp, scale=0.125)
                                if mband is not None:
                                    m, mc = mband
                                    pr = p2s.tile([P, P], bf16,
                                                  tag=f"pr{h % 2}",
                                                  name=f"pr{h % 2}")
                                    nc.vector.tensor_mul(
                                        out=pr[:], in0=pt[:, mc:mc + P],
                                        in1=bm[:, m, :])
                                    nc.vector.tensor_copy(
                                        out=pt[:, mc:mc + P], in_=pr[:])
                                pts[h] = pt
                            if prev is not None:
                                pcr, pw, pb, ppts = prev
                                for h in heads:
                                    nc.tensor.matmul(
                                        pvs[h][:, pcr:pcr + pw],
                                        vts[:, pb, h, :],
                                        ppts[h][:, pcr:pcr + pw],
                                        start=(pb == 0),
                                        stop=(pb == kmax - 1),
                                        skip_group_check=True,
                                    )
                            prev = (cr, w, kc, pts)
                        pcr, pw, pb, ppts = prev
                        for h in heads:
                            nc.tensor.matmul(
                                pvs[h][:, pcr:pcr + pw],
                                vts[:, pb, h, :],
                                ppts[h][:, pcr:pcr + pw],
                                start=(pb == 0), stop=(pb == kmax - 1),
                                skip_group_check=True,
                            )
                        # normalization: DVE part now, PE broadcast deferred
                        # into the next pass's kc loop
                        for h in heads:
                            den = p2s.tile([1, QC], f32, tag=f"den{h % 2}")
                            nc.vector.tensor_copy(out=den[:],
                                                  in_=pvs[h][PD:PD + 1, :])
                            recsb = p2s.tile([1, QC], f32, tag=f"rec{h % 2}")
                            nc.vector.reciprocal_approx_fast(
                                out=recsb[:], in_=den[:])
                            recb = p2s.tile([1, QC], bf16, tag=f"recb{h % 2}")
                            nc.vector.tensor_copy(out=recb[:], in_=recsb[:])

                            def normB(h=h, recb=recb, pvs_h=pvs[h]):
                                bcp = qkps.tile([P, QC], f32, tag="qk",
                                                name="bcp")
                                nc.tensor.matmul(
                                    bcp[0:PD, :], ones_col[:], recb[:],
                                    start=True, stop=True,
                                )
                                bcs = p2s.tile([PD, QC], f32, tag="bcs",
                                               name="bcs")
                                nc.vector.tensor_copy(out=bcs[:],
                                                      in_=bcp[0:PD, :])
                                po = (h % 2) * PD
                                nc.vector.tensor_mul(
                                    out=attnT[po:po + PD, h // 2, :],
                                    in0=pvs_h[0:PD, :],
                                    in1=bcs[:],
                                )
                            pending_normB.append(normB)
                    for fn in pending_normB:
                        fn()

                    # output projection for this q-chunk
                    for b2 in range(QC // P):
                        psf = fps.tile([P, D], f32, tag="fin")
                        for dc in range(DC):
                            nc.tensor.matmul(
                                psf[:],
                                attnT[:, dc, b2 * P:(b2 + 1) * P],
                                wo[:, dc, :],
                                start=(dc == 0), stop=(dc == DC - 1),
                            )
                        osb = p2s.tile([P, D], f32, tag="osb")
                        nc.vector.tensor_add(out=osb[:], in0=psf[:], in1=bo[:])
                        nc.sync.dma_start(
                            out_d[qc * QC + b2 * P:qc * QC + (b2 + 1) * P, :],
                            osb[:],
                        )
    nc.finalize()
    return nc


def _build_generic(mode: str):
    """Fallback build for non-causal masks (none / binary / additive)."""
    import concourse.mybir as mybir
    import concourse.tile as tile
    from concourse import bacc
    from concourse.masks import make_identity

    f32 = mybir.dt.float32
    f32r = mybir.dt.float32r
    bf16 = mybir.dt.bfloat16
    Exp = mybir.ActivationFunctionType.Exp
    Alu = mybir.AluOpType

    GQC = 512         # generic path q-chunk
    GNQC = QR // GQC

    nc = bacc.Bacc(debug=False, target_bir_lowering=False)

    xb = nc.declare_dram_parameter("xb", [S, D], f32, isOutput=False)
    xq = nc.declare_dram_parameter("xq", [QR, D], f32, isOutput=False)
    wq_d = nc.declare_dram_parameter("wq", [P, DC, D], f32r, isOutput=False)
    wk_d = nc.declare_dram_parameter("wk", [P, DC, D], f32r, isOutput=False)
    wv_d = nc.declare_dram_parameter("wv", [P, DC, D], f32r, isOutput=False)
    wo_d = nc.declare_dram_parameter("wo", [P, DC, D], f32r, isOutput=False)
    bq_d = nc.declare_dram_parameter("bq", [P, DC], f32, isOutput=False)
    bk_d = nc.declare_dram_parameter("bk", [P, DC], f32, isOutput=False)
    bv_d = nc.declare_dram_parameter("bv", [P, D], f32, isOutput=False)
    bo_d = nc.declare_dram_parameter("bo", [P, D], f32, isOutput=False)
    ones_d = nc.declare_dram_parameter("ones", [P, H], bf16, isOutput=False)
    onesr_d = nc.declare_dram_parameter("onesr", [1, PD], f32r, isOutput=False)
    if mode == "add":
        maskT_d = nc.declare_dram_parameter("maskT", [S, QR], f32, isOutput=False)
    elif mode == "bin":
        maskT_d = nc.declare_dram_parameter("maskT", [S, QR], bf16, isOutput=False)
    out_d = nc.declare_dram_parameter("out", [QR, D], f32, isOutput=True)

    with tile.TileContext(nc) as tc, nc.allow_low_precision(
            reason="float32r tiles are 4-byte fp32; PE rounds reads only"):
        with (
            tc.tile_pool(name="const", bufs=1) as constp,
            tc.tile_pool(name="kt", bufs=1) as ktp,
            tc.tile_pool(name="vt", bufs=1) as vtp,
            tc.tile_pool(name="work", bufs=3) as work,
        ):
            ident = constp.tile([P, P], f32, tag="ident")
            make_identity(nc, ident)
            ones_col = constp.tile([1, PD], f32r, tag="ones")
            nc.sync.dma_start(ones_col[:], onesr_d[:])

            wq = constp.tile([P, DC, D], f32r, tag="wq")
            wo = constp.tile([P, DC, D], f32r, tag="wo")
            bq = constp.tile([P, DC], f32, tag="bq")
            bo = constp.tile([P, D], f32, tag="bo")
            for sb_t, dr_t in [(wq, wq_d), (wo, wo_d), (bq, bq_d), (bo, bo_d)]:
                nc.sync.dma_start(sb_t[:], dr_t[:])

            kts = [ktp.tile([P, HP, SC], bf16, tag=f"kt{i}", name=f"kt{i}")
                   for i in range(NSC)]
            vts = [vtp.tile([P, H, PD + 1], bf16, tag=f"v{i}", name=f"v{i}")
                   for i in range(NKT)]
            for t in vts:
                nc.sync.dma_start(t[:, :, PD:PD + 1], ones_d[:, :, None])

            with (
                tc.tile_pool(name="p1w", bufs=1) as p1w,
                tc.tile_pool(name="ps1", bufs=2, space="PSUM") as ps1,
            ):
                wk = p1w.tile([P, DC, D], f32r, tag="wk")
                wv = p1w.tile([P, DC, D], f32r, tag="wv")
                bk = p1w.tile([P, DC], f32, tag="bk")
                bv = p1w.tile([P, D], f32, tag="bv")
                for sb_t, dr_t in [(wk, wk_d), (wv, wv_d), (bk, bk_d), (bv, bv_d)]:
                    nc.sync.dma_start(sb_t[:], dr_t[:])

                for sci in range(NSC):
                    xraw = work.tile([P, SC // P, D], f32, tag="xraw")
                    nc.sync.dma_start(
                        xraw[:],
                        xb[sci * SC:(sci + 1) * SC].rearrange(
                            "(rt p) d -> p rt d", p=P),
                    )
                    xt = work.tile([P, DC, SC], f32r, tag="xt")
                    for rt in range(SC // P):
                        pst = ps1.tile([P, D], f32, tag="tps")
                        for dc in range(DC):
                            nc.tensor.transpose(
                                pst[:, dc * P:(dc + 1) * P],
                                xraw[:, rt, dc * P:(dc + 1) * P],
                                ident[:],
                            )
                        nc.scalar.copy(
                            out=xt[:, :, rt * P:(rt + 1) * P],
                            in_=pst[:].rearrange("p (dc j) -> p dc j", dc=DC),
                        )
                    for hp in range(HP):
                        psk = ps1.tile([P, SC], f32, tag="kproj")
                        for dc in range(DC):
                            nc.tensor.matmul(
                                psk[:],
                                wk[:, dc, hp * P:(hp + 1) * P],
                                xt[:, dc, :],
                                start=(dc == 0), stop=(dc == DC - 1),
                            )
                        nc.scalar.add(kts[sci][:, hp, :], psk[:], bk[:, hp:hp + 1])
                    for rt in range(SC // P):
                        psv = ps1.tile([P, D], f32, tag="vproj")
                        for dc in range(DC):
                            nc.tensor.matmul(
                                psv[:],
                                xt[:, dc, rt * P:(rt + 1) * P],
                                wv[:, dc, :],
                                start=(dc == 0), stop=(dc == DC - 1),
                            )
                        kti = sci * (SC // P) + rt
                        nc.vector.tensor_add(
                            out=vts[kti][:, :, 0:PD],
                            in0=psv[:].rearrange("p (h d) -> p h d", h=H),
                            in1=bv[:].rearrange("p (h d) -> p h d", h=H),
                        )

            with (
                tc.tile_pool(name="p2", bufs=2) as p2,
                tc.tile_pool(name="p2s", bufs=3) as p2s,
                tc.tile_pool(name="p2a", bufs=1) as p2a,
                tc.tile_pool(name="qkps", bufs=3, space="PSUM") as qkps,
                tc.tile_pool(name="pvps", bufs=1, space="PSUM") as pvps,
                tc.tile_pool(name="fps", bufs=1, space="PSUM") as fps,
            ):
                for qc in range(GNQC):
                    xqraw = work.tile([P, GQC // P, D], f32, tag="xraw")
                    nc.sync.dma_start(
                        xqraw[:],
                        xq[qc * GQC:(qc + 1) * GQC].rearrange(
                            "(rt p) d -> p rt d", p=P),
                    )
                    xqt = work.tile([P, DC, GQC], f32r, tag="xt")
                    for rt in range(GQC // P):
                        pst = qkps.tile([P, D], f32, tag="qk")
                        for dc in range(DC):
                            nc.tensor.transpose(
                                pst[:, dc * P:(dc + 1) * P],
                                xqraw[:, rt, dc * P:(dc + 1) * P],
                                ident[:],
                            )
                        nc.scalar.copy(
                            out=xqt[:, :, rt * P:(rt + 1) * P],
                            in_=pst[:].rearrange("p (dc j) -> p dc j", dc=DC),
                        )
                    qt = p2.tile([P, HP, GQC], bf16, tag="qt")
                    for hp in range(HP):
                        psq = qkps.tile([P, D], f32, tag="qk")
                        for dc in range(DC):
                            nc.tensor.matmul(
                                psq[:, 0:GQC],
                                wq[:, dc, hp * P:(hp + 1) * P],
                                xqt[:, dc, :],
                                start=(dc == 0), stop=(dc == DC - 1),
                            )
                        nc.scalar.add(qt[:, hp, :], psq[:, 0:GQC], bq[:, hp:hp + 1])

                    attnT = p2a.tile([P, DC, GQC], f32r, tag="attnT")
                    for hg in range(NHG):
                        heads = range(hg * HG, (hg + 1) * HG)
                        pvs = {h: pvps.tile([PD + 1, GQC], f32, tag=f"pv{h % HG}",
                                            name=f"pv{h}")
                               for h in heads}
                        for kc in range(NKT):
                            if mode == "add":
                                mt = p2s.tile([P, GQC], f32, tag="mt")
                            elif mode == "bin":
                                mt = p2s.tile([P, GQC], bf16, tag="mt")
                            if mode != "none":
                                nc.sync.dma_start(
                                    mt[:],
                                    maskT_d[kc * P:(kc + 1) * P,
                                            qc * GQC:(qc + 1) * GQC],
                                )
                            for h in heads:
                                po = (h % 2) * PD
                                pss = qkps.tile([P, D], f32, tag="qk")
                                nc.tensor.matmul(
                                    pss[:, 0:GQC],
                                    kts[kc // (SC // P)][
                                        po:po + PD, h // 2,
                                        (kc % (SC // P)) * P:
                                        (kc % (SC // P) + 1) * P],
                                    qt[po:po + PD, h // 2, :],
                                    start=True, stop=True,
                                )
                                pt = p2s.tile([P, GQC], bf16, tag="pt")
                                if mode == "add":
                                    st = p2s.tile([P, GQC], f32, tag="st")
                                    nc.vector.scalar_tensor_tensor(
                                        out=st[:], in0=mt[:], scalar=-1e9,
                                        in1=pss[:, 0:GQC],
                                        op0=Alu.mult, op1=Alu.add,
                                    )
                                    nc.scalar.activation(pt[:], st[:], Exp,
                                                         scale=0.125)
                                elif mode == "bin":
                                    pr = p2s.tile([P, GQC], bf16, tag="pr")
                                    nc.scalar.activation(pr[:], pss[:, 0:GQC], Exp,
                                                         scale=0.125)
                                    nc.vector.tensor_mul(
                                        out=pt[:], in0=pr[:], in1=mt[:])
                                else:
                                    nc.scalar.activation(pt[:], pss[:, 0:GQC], Exp,
                                                         scale=0.125)
                                nc.tensor.matmul(
                                    pvs[h][:],
                                    vts[kc][:, h, :],
                                    pt[:],
                                    start=(kc == 0), stop=(kc == NKT - 1),
                                    skip_group_check=True,
                                )
                        for h in heads:
                            recip = p2s.tile([1, GQC], f32r, tag="recip")
                            nc.vector.reciprocal(recip[:], pvs[h][PD:PD + 1, :])
                            bcp = fps.tile([PD, GQC], f32, tag="fin")
                            nc.tensor.matmul(
                                bcp[:], ones_col[:], recip[:],
                                start=True, stop=True,
                            )
                            bcs = p2s.tile([PD, GQC], f32, tag="bcs")
                            nc.vector.tensor_copy(out=bcs[:], in_=bcp[:])
                            po = (h % 2) * PD
                            nc.vector.tensor_mul(
                                out=attnT[po:po + PD, h // 2, :],
                                in0=pvs[h][0:PD, :],
                                in1=bcs[:],
                            )

                    for rt in range(GQC // P):
                        psf = fps.tile([P, D], f32, tag="fin")
                        for dc in range(DC):
                            nc.tensor.matmul(
                                psf[:],
                                attnT[:, dc, rt * P:(rt + 1) * P],
                                wo[:, dc, :],
                                start=(dc == 0), stop=(dc == DC - 1),
                            )
                        osb = p2s.tile([P, D], f32, tag="osb")
                        nc.vector.tensor_add(out=osb[:], in0=psf[:], in1=bo[:])
                        nc.sync.dma_start(
                            out_d[qc * GQC + rt * P: qc * GQC + (rt + 1) * P, :],
                            osb[:],
                        )
    nc.finalize()
    return nc


def _get_prog(mode: str):
    if mode not in _prog_cache:
        _prog_cache[mode] = (_build_tril() if mode == "tril"
                             else _build_generic(mode))
    return _prog_cache[mode]


def _q_rows(c, mode):
    """Query rows (into this core's batch) owned by core c."""
    if mode == "tril":
        # interleaved 128-row blocks so the causal kv range per q-chunk is
        # identical on every core
        j = np.arange(QR // P)
        base = (j * CPB + (c % CPB)) * P
        return (base[:, None] + np.arange(P)[None, :]).ravel()
    r0 = (c % CPB) * QR
    return np.arange(r0, r0 + QR)


def _warr(W, dtype):
    return np.ascontiguousarray(
        np.asarray(W, dtype=np.float32).reshape(DC, P, D)
        .transpose(1, 0, 2)).astype(dtype)


def _barr(b):
    return np.ascontiguousarray(
        np.asarray(b, dtype=np.float32).reshape(DC, P).T)


def make_in_maps(inputs, mask, Wq, bq, Wk, bk, Wv, bv, Wo, bo):
    import ml_dtypes
    bf = ml_dtypes.bfloat16
    inputs = np.asarray(inputs, dtype=np.float32)
    mask = np.asarray(mask, dtype=np.float32)
    if np.array_equal(mask, np.triu(np.ones((S, S), dtype=np.float32), 1)):
        mode = "tril"
    elif not np.any(mask):
        mode = "none"
    elif bool(((mask == 0.0) | (mask == 1.0)).all()):
        mode = "bin"
    else:
        mode = "add"

    in_maps = []
    if mode == "tril":
        shared = {
            "wq": _warr(Wq, bf), "wk": _warr(Wk, bf), "wv": _warr(Wv, bf),
            "wo": _warr(Wo, bf),
            "bq": _barr(bq), "bk": _barr(bk),
            "bv": np.ascontiguousarray(
                np.broadcast_to(np.asarray(bv, dtype=np.float32), (P, D))),
            "bo": np.ascontiguousarray(
                np.broadcast_to(np.asarray(bo, dtype=np.float32), (P, D))),
        }
        tri = np.triu(np.ones((P, P), dtype=np.float32))  # keep k <= q
        for c in range(NCORES):
            cl = c % CPB
            bmask = np.zeros((P, 4, P), dtype=np.float32)
            for m in range(4):
                if m < cl:
                    bmask[:, m, :] = 1.0
                elif m == cl:
                    bmask[:, m, :] = tri
            m = dict(shared)
            m["bandmask"] = np.ascontiguousarray(bmask).astype(bf)
            m["ones"] = np.ones((P, NKT, H), dtype=bf)
            bidx = c // CPB
            m["xb"] = np.ascontiguousarray(inputs[bidx])
            m["xq"] = np.ascontiguousarray(inputs[bidx][_q_rows(c, mode)])
            in_maps.append(m)
        return mode, in_maps

    # generic path (fp32/f32r)
    if mode == "none":
        maskT = None
    elif mode == "bin":
        maskT = np.ascontiguousarray(1.0 - mask.T).astype(bf)
    else:
        maskT = np.ascontiguousarray(mask.T)
    shared = {
        "wq": _warr(Wq, np.float32), "wk": _warr(Wk, np.float32),
        "wv": _warr(Wv, np.float32), "wo": _warr(Wo, np.float32),
        "bq": _barr(bq), "bk": _barr(bk),
        "bv": np.ascontiguousarray(
            np.broadcast_to(np.asarray(bv, dtype=np.float32), (P, D))),
        "bo": np.ascontiguousarray(
            np.broadcast_to(np.asarray(bo, dtype=np.float32), (P, D))),
    }
    for c in range(NCORES):
        b = c // CPB
        rows = _q_rows(c, mode)
        m = dict(shared)
        m["ones"] = np.ones((P, H), dtype=bf)
        m["onesr"] = np.ones((1, PD), dtype=np.float32)
        m["xb"] = np.ascontiguousarray(inputs[b])
        m["xq"] = np.ascontiguousarray(inputs[b][rows])
        if maskT is not None:
            m["maskT"] = np.ascontiguousarray(maskT[:, rows])
        in_maps.append(m)
    return mode, in_maps


def assemble(results, mode):
    out = np.empty((B, S, D), dtype=np.float32)
    for c in range(NCORES):
        b = c // CPB
        out[b, _q_rows(c, mode)] = results[c]["out"]
    return out


def kernel(inputs, mask, Wq, bq, Wk, bk, Wv, bv, Wo, bo):
    from concourse.bass_utils import run_bass_kernel_spmd

    mode, in_maps = make_in_maps(inputs, mask, Wq, bq, Wk, bk, Wv, bv, Wo, bo)
    nc = _get_prog(mode)
    res = run_bass_kernel_spmd(nc, in_maps, core_ids=list(range(NCORES)))
    return assemble(res.results, mode)
